# revision 13
# baseline (speedup 1.0000x reference)
"""Trainium2 Bass kernel for nn_Attention_7078106104284.

Self-attention block (SAGAN-style) over x[8, 256, 64, 64]:
  q = wq@x+bq [32,n], k = wk@x+bk [32,n], v = wv@x+bv [256,n], n = 4096
  attn = softmax(q^T k, axis=m);  y = x + gamma * (v @ attn^T)

Sharding: data-parallel over batch - one batch element per NeuronCore (8 cores).

Numerics: all matmuls bf16; q/k use a hi/lo split (a ~ a_hi + a_lo, both bf16)
so logits carry ~16 mantissa bits via K=96 stacked strips
  q_stack = [q_hi; q_hi; q_lo], k_stack = [k_lo; k_hi; k_hi].
Softmax max-subtraction skipped (|logit| < 50 << 88). Bias algebra:
  - bk drops entirely (q.bk is constant per softmax row -> cancels),
  - bq fuses into the q_hi evacuation on ACT,
  - bv folds into the residual (sum_m attn = 1), precomputed on host.

Dataflow per core (DMA count is minimized - each dma_start costs ~0.6us on
the issuing sequencer's HWDGE slot):
  - x arrives hi+lo packed, in 8 chunk DMAs; q/k projections start on chunk 0
    while later chunks stream (no startup DMA wall).
  - q+k share one stationary [128, 64]: each projection term is ONE matmul
    into pqk[64, 512] (q rows 0:32, k rows 32:64). Evac: ACT writes the hi
    strips (bq fused), DVE writes lo strips via fused (pqk+b)-hi.
  - vT[m, c'] via stationary x_hi chunks, interleaved with projections;
    Z ones-column via one strided memset (no K=1 matmuls).
  - strip duplication = 4 whole-row SBUF->SBUF DMAs after the loop.
  - attention: 16 rounds/group of (2 logit matmuls K=96 -> one fused exp on
    ACT over [128, 2*512] bf16 out -> 8 AV matmuls), plt double-buffered so
    logits(j+1) overlap exp(j); AV accumulates uT[n-sub, c'] in PSUM.
  - epilogue per sub: DVE normalize (gamma/Z), GPSIMD adds the residual
    (x^T + gamma*bv, bf16), one merged y DMA per group in [N, C] layout
    (host transposes back). No PE transposes anywhere.
"""

import sys

sys.path.insert(0, "/opt/trn_rl_repo")

import numpy as np
from contextlib import ExitStack

import concourse.bass as bass
import concourse.bacc as bacc
import concourse.tile as tile
import concourse.mybir as mybir
from concourse.bass_utils import run_bass_kernel_spmd

dt = mybir.dt
AF = mybir.ActivationFunctionType
ALU = mybir.AluOpType

B = 8
C = 256
C8 = 32
N = 4096          # h*w spatial positions
NG = 512          # n-group width (one PSUM bank of fp32)
G = N // NG       # 8 n-groups
MC = N // 128     # 32 m-chunks
EW = 2            # m-chunks per exp batch (PSUM banks per plt buffer)
RND = MC // EW    # 16 rounds per group
CP = C + 1        # AV output channels incl. the Z ones-column


def build_program(reps=1, ablate=()):
    nc = bacc.Bacc("TRN2", target_bir_lowering=False)
    f32 = dt.float32
    bf16 = dt.bfloat16
    # x_hl: [hl, C, N] = x_hi (hl=0) and x_lo (hl=1) packed in one tensor
    xhl_d = nc.declare_dram_parameter("x_hl", [2 * C, N], bf16, isOutput=False)
    xr_d = nc.declare_dram_parameter("x_res", [N, C], bf16, isOutput=False)
    # packed q|k weights, hi/lo stacked: [2C, 64] = [[wqT_hi wkT_hi]; [lo]]
    wqk_d = nc.declare_dram_parameter("wqkT_hl", [2 * C, 64], bf16, isOutput=False)
    wv_d = nc.declare_dram_parameter("wvT_h", [C, C], bf16, isOutput=False)
    bq_d = nc.declare_dram_parameter("bq", [C8, 1], f32, isOutput=False)
    gamma_d = nc.declare_dram_parameter("gamma", [1, 1], f32, isOutput=False)
    y_d = nc.declare_dram_parameter("y", [N, C], bf16, isOutput=True)

    with tile.TileContext(nc) as tc, ExitStack() as ctx:
        sing = ctx.enter_context(tc.tile_pool(name="sing", bufs=1))
        # e tiles live a full group (RND rounds) before AV consumes them
        epool = ctx.enter_context(tc.tile_pool(name="epool", bufs=RND + 2))
        ypool = ctx.enter_context(tc.tile_pool(name="ypool", bufs=2))
        scal = ctx.enter_context(tc.tile_pool(name="scal", bufs=4))

        lt_ps = ctx.enter_context(tc.tile_pool(name="lt_ps", bufs=2, space="PSUM"))
        u_ps = ctx.enter_context(tc.tile_pool(name="u_ps", bufs=1, space="PSUM"))

        for _rep in range(reps):
            # chunk 0 of x first (unblocks projections), then weights, then
            # the rest of x. slots (hl cc): 0=hi0, 1=hi1, 2=lo0, 3=lo1
            xhl_view = xhl_d[:].rearrange("(hl cc p) m -> p (hl cc) m", p=128, hl=2)
            xhl_t = []
            for s in range(G):
                t = sing.tile([128, 4, NG], bf16)
                nc.sync.dma_start(out=t, in_=xhl_view[:, :, s * NG:(s + 1) * NG])
                xhl_t.append(t)
                if s == 0:
                    wqk_sb = sing.tile([128, 4, 64], bf16)
                    nc.sync.dma_start(
                        out=wqk_sb,
                        in_=wqk_d[:].rearrange("(hl cc p) o -> p (hl cc) o",
                                               p=128, hl=2))
                    wv_sb = sing.tile([128, 2, C], bf16)
                    nc.sync.dma_start(
                        out=wv_sb, in_=wv_d[:].rearrange("(cc p) c -> p cc c", p=128))
                    bq_sb = sing.tile([C8, 1], f32)
                    nc.sync.dma_start(out=bq_sb, in_=bq_d[:])
                    g128 = sing.tile([128, 1], f32)
                    nc.sync.dma_start(
                        out=g128,
                        in_=bass.AP(tensor=gamma_d, offset=0, ap=[[0, 128], [1, 1]]),
                    )

            # ---- q/k projections + vT, per chunk ----
            q_stack = sing.tile([3 * C8, N], bf16)
            k_stack = sing.tile([3 * C8, N], bf16)
            qlo_full = sing.tile([C8, N], bf16)
            klo_full = sing.tile([64, N], bf16)
            vt_sb = sing.tile([128, MC, CP], bf16)
            nc.vector.memset(vt_sb[:, :, C:CP], 1.0)   # Z ones-column
            term_slots = [0, 1, 2, 3, 0, 1]            # x slot per term
            w_slots = [0, 1, 0, 1, 2, 3]               # w slot per term
            for s in range(G):
                sl = slice(s * NG, (s + 1) * NG)
                pqk = u_ps.tile([64, NG], f32, tag=f"u{s % 2}", name="pqk")
                nterm = len(w_slots) if "lo" not in ablate else 2
                for i in range(nterm):
                    nc.tensor.matmul(pqk, wqk_sb[:, w_slots[i], :],
                                     xhl_t[s][:, term_slots[i], :],
                                     start=(i == 0), stop=(i == nterm - 1))
                # hi strips via ACT (bias fused for q); lo strips via one
                # fused DVE op each: lo = (pqk + b) - hi
                nc.scalar.activation(q_stack[0:C8, sl], pqk[0:C8, :],
                                     AF.Identity, bias=bq_sb)
                nc.scalar.activation(k_stack[C8:2 * C8, sl], pqk[C8:64, :], AF.Copy)
                nc.vector.scalar_tensor_tensor(
                    qlo_full[:, sl], pqk[0:C8, :], bq_sb, q_stack[0:C8, sl],
                    ALU.add, ALU.subtract)
                nc.vector.scalar_tensor_tensor(
                    klo_full[C8:64, sl], pqk[C8:64, :], 0.0,
                    k_stack[C8:2 * C8, sl], ALU.add, ALU.subtract)

                # vT for this chunk's 4 m-chunks (copies split DVE/ACT)
                for mc in range(4 * s, 4 * s + 4):
                    msl = slice((mc % 4) * 128, (mc % 4 + 1) * 128)
                    pv = u_ps.tile([128, C], f32, tag=f"u{2 + mc % 2}", name="pv")
                    for cc in range(2):
                        nc.tensor.matmul(pv, xhl_t[s][:, cc, msl], wv_sb[:, cc, :],
                                         start=(cc == 0), stop=(cc == 1))
                    if mc % 2 == 0:
                        nc.vector.tensor_copy(vt_sb[:, mc, 0:C], pv)
                    else:
                        nc.scalar.activation(vt_sb[:, mc, 0:C], pv, AF.Copy)

            # strip duplication: 4 whole-row partition-shifting DMAs
            nc.sync.dma_start(out=q_stack[C8:2 * C8, :], in_=q_stack[0:C8, :])
            nc.sync.dma_start(out=q_stack[2 * C8:3 * C8, :], in_=qlo_full)
            nc.sync.dma_start(out=k_stack[2 * C8:3 * C8, :],
                              in_=k_stack[C8:2 * C8, :])
            nc.sync.dma_start(out=k_stack[0:C8, :], in_=klo_full[C8:64, :])

            # residual chunks (first needed ~20us into attention)
            xr_t = []
            for h in range(2):
                tr = sing.tile([128, 16, C], bf16)
                nc.sync.dma_start(
                    out=tr,
                    in_=xr_d[:].rearrange("(gs p) c -> p gs c", p=128)[
                        :, h * 16:(h + 1) * 16, :])
                xr_t.append(tr)

            # ---- attention, software-pipelined one group deep ----
            e_tiles = {}
            u_tiles = {}

            def issue_lt_exp(g, j):
                sl = slice(g * NG, (g + 1) * NG)
                plt = lt_ps.tile([128, EW, NG], f32, tag="plt", name="plt")
                for rg in range(EW if "lt" not in ablate else 1):
                    mc = EW * j + rg
                    msl = slice(mc * 128, (mc + 1) * 128)
                    nc.tensor.matmul(plt[:, rg, :], k_stack[:, msl], q_stack[:, sl],
                                     start=True, stop=True)
                e_t = epool.tile([128, EW, NG], bf16, tag="e", name="e_t")
                fn = AF.Exp if "exp" not in ablate else AF.Copy
                nc.scalar.activation(e_t, plt, fn)
                e_tiles[(g, j)] = e_t

            def issue_av(g, j):
                uts = u_tiles[g]
                e_t = e_tiles.pop((g, j))
                if "av" in ablate:
                    if j == 0:
                        for sub in range(4):
                            nc.tensor.matmul(uts[sub],
                                             e_t[:, 0, sub * 128:(sub + 1) * 128],
                                             vt_sb[:, 0, :], start=True, stop=True)
                    return
                for rg in range(EW):
                    mc = EW * j + rg
                    first = (j == 0 and rg == 0)
                    last = (j == RND - 1 and rg == EW - 1)
                    for sub in range(4):
                        nc.tensor.matmul(uts[sub],
                                         e_t[:, rg, sub * 128:(sub + 1) * 128],
                                         vt_sb[:, mc, :],
                                         start=first, stop=last)

            def issue_epilogue(g):
                # DVE normalizes (gamma/Z); GPSIMD adds the bf16 residual
                uts = u_tiles.pop(g)
                y_g = ypool.tile([128, 4, C], bf16, tag="yg", name="y_g")
                for sub in range(4):
                    ut = uts[sub]
                    rinv = scal.tile([128, 1], f32, tag="rinv", name="rinv")
                    nc.vector.reciprocal(rinv, ut[:, C:CP])
                    gsc = scal.tile([128, 1], f32, tag="gsc", name="gsc")
                    nc.vector.tensor_scalar_mul(gsc, rinv, g128)
                    ysc = scal.tile([128, C], f32, tag="ysc", name="ysc")
                    nc.vector.tensor_scalar_mul(ysc, ut[:, 0:C], gsc)
                    nc.gpsimd.tensor_add(y_g[:, sub, :], ysc,
                                         xr_t[g // 4][:, (g % 4) * 4 + sub, :])
                nc.sync.dma_start(
                    out=y_d[:].rearrange("(gs p) c -> p gs c", p=128)[
                        :, g * 4:(g + 1) * 4, :],
                    in_=y_g,
                )

            for g in range(G + 1):
                if g < G:
                    u_tiles[g] = [u_ps.tile([128, CP], f32, tag=f"u{s}", name=f"u{s}")
                                  for s in range(4)]
                for j in range(RND):
                    if g < G:
                        issue_lt_exp(g, j)
                    if g >= 1:
                        issue_av(g - 1, j)
                if g >= 1:
                    issue_epilogue(g - 1)

    nc.compile()
    return nc


def prepare_in_maps(inputs):
    """Host-side prep: hi/lo splits, packed weights, residual fold."""
    import ml_dtypes
    bf = ml_dtypes.bfloat16
    x = np.asarray(inputs["x"], dtype=np.float32)
    wq = np.asarray(inputs["wq"], dtype=np.float32)
    bq = np.asarray(inputs["bq"], dtype=np.float32)
    wk = np.asarray(inputs["wk"], dtype=np.float32)
    wv = np.asarray(inputs["wv"], dtype=np.float32)
    bv = np.asarray(inputs["bv"], dtype=np.float32)
    gamma = np.asarray(inputs["gamma"], dtype=np.float32)

    xr = np.ascontiguousarray(x.reshape(B, C, N))
    x_hi = xr.astype(bf)
    x_lo = (xr - x_hi.astype(np.float32)).astype(bf)
    x_hl = np.concatenate([x_hi, x_lo], axis=1)  # [B, 2C, N]
    # residual in [N, C] layout with gamma*bv folded in (bf16: |err| ~0.4%)
    xres = np.ascontiguousarray(
        xr.transpose(0, 2, 1) + gamma[0] * bv[None, None, :]).astype(bf)

    wqkT = np.concatenate([wq.T, wk.T], axis=1)  # [C, 64]
    hi = wqkT.astype(bf)
    lo = (wqkT - hi.astype(np.float32)).astype(bf)
    wqkT_hl = np.ascontiguousarray(np.concatenate([hi, lo], axis=0))

    shared = {
        "wqkT_hl": wqkT_hl,
        "wvT_h": np.ascontiguousarray(wv.T.astype(bf)),
        "bq": np.ascontiguousarray(bq.reshape(C8, 1)),
        "gamma": np.ascontiguousarray(gamma.reshape(1, 1)),
    }
    return [dict(shared,
                 x_hl=np.ascontiguousarray(x_hl[i]),
                 x_res=xres[i]) for i in range(B)]


_nc_cache = None


def kernel(**inputs) -> np.ndarray:
    global _nc_cache
    if _nc_cache is None:
        _nc_cache = build_program()
    nc = _nc_cache

    in_maps = prepare_in_maps(inputs)
    res = run_bass_kernel_spmd(nc, in_maps, core_ids=list(range(B)))
    # y comes back [N, C] bf16 per core; transpose to [C, N] on host
    y = np.stack([res.results[i]["y"].astype(np.float32).T for i in range(B)],
                 axis=0)
    return np.ascontiguousarray(y.reshape(B, C, 64, 64))


if __name__ == "__main__":
    rng = np.random.default_rng(0)
    ins = {
        "x": rng.standard_normal((B, C, 64, 64), dtype=np.float32),
        "wq": rng.standard_normal((C8, C), dtype=np.float32) / 16,
        "bq": rng.standard_normal((C8,), dtype=np.float32) * 0.01,
        "wk": rng.standard_normal((C8, C), dtype=np.float32) / 16,
        "bk": rng.standard_normal((C8,), dtype=np.float32) * 0.01,
        "wv": rng.standard_normal((C, C), dtype=np.float32) / 16,
        "bv": rng.standard_normal((C,), dtype=np.float32) * 0.01,
        "gamma": rng.standard_normal((1,), dtype=np.float32) * 0.1,
    }
    out = kernel(**ins)
    print("kernel output", out.shape, out.dtype)


# revision 19
# speedup vs baseline: 1.0733x; 1.0733x over previous
"""Trainium2 Bass kernel for nn_Attention_7078106104284.

Self-attention block (SAGAN-style) over x[8, 256, 64, 64]:
  q = wq@x+bq [32,n], k = wk@x+bk [32,n], v = wv@x+bv [256,n], n = 4096
  attn = softmax(q^T k, axis=m);  y = x + gamma * (v @ attn^T)

Sharding: data-parallel over batch - one batch element per NeuronCore (8 cores).

Numerics: all matmuls bf16; q/k use a hi/lo split (a ~ a_hi + a_lo, both bf16)
so logits carry ~16 mantissa bits via K=96 stacked strips
  q_stack = [q_hi; q_hi; q_lo], k_stack = [k_lo; k_hi; k_hi].
Softmax max-subtraction skipped (|logit| < 50 << 88). Bias algebra:
  - bk drops entirely (q.bk is constant per softmax row -> cancels),
  - bq fuses into the q_hi evacuation on ACT,
  - bv folds into the residual (sum_m attn = 1), precomputed on host.

Dataflow per core (DMA count is minimized - each dma_start costs ~0.6us on
the issuing sequencer's HWDGE slot):
  - x arrives hi+lo packed, in 8 chunk DMAs; q/k projections start on chunk 0
    while later chunks stream (no startup DMA wall).
  - q+k share one stationary [128, 64]: each projection term is ONE matmul
    into pqk[64, 512] (q rows 0:32, k rows 32:64). Evac: ACT writes the hi
    strips (bq fused), DVE writes lo strips via fused (pqk+b)-hi.
  - vT[m, c'] via stationary x_hi chunks, interleaved with projections;
    Z ones-column via one strided memset (no K=1 matmuls).
  - strip duplication = 4 whole-row SBUF->SBUF DMAs after the loop.
  - attention: 16 rounds/group of (2 logit matmuls K=96 -> one fused exp on
    ACT over [128, 2*512] bf16 out -> 8 AV matmuls), plt double-buffered so
    logits(j+1) overlap exp(j); AV accumulates uT[n-sub, c'] in PSUM.
  - epilogue per sub: DVE normalize (gamma/Z), GPSIMD adds the residual
    (x^T + gamma*bv, bf16), one merged y DMA per group in [N, C] layout
    (host transposes back). No PE transposes anywhere.
"""

import sys

sys.path.insert(0, "/opt/trn_rl_repo")

import numpy as np
from contextlib import ExitStack

import concourse.bass as bass
import concourse.bacc as bacc
import concourse.tile as tile
import concourse.mybir as mybir
from concourse.bass_utils import run_bass_kernel_spmd

dt = mybir.dt
AF = mybir.ActivationFunctionType
ALU = mybir.AluOpType

B = 8
C = 256
C8 = 32
N = 4096          # h*w spatial positions
NG = 512          # n-group width (one PSUM bank of fp32)
G = N // NG       # 8 n-groups
MC = N // 128     # 32 m-chunks
EW = 2            # m-chunks per exp batch (PSUM banks per plt buffer)
RND = MC // EW    # 16 rounds per group
CP = C + 1        # AV output channels incl. the Z ones-column


def build_program(reps=1, ablate=(), dma_mode="split2"):
    nc = bacc.Bacc("TRN2", target_bir_lowering=False)
    f32 = dt.float32
    bf16 = dt.bfloat16
    # x_hl: [hl, C, N] = x_hi (hl=0) and x_lo (hl=1) packed in one tensor
    xhl_d = nc.declare_dram_parameter("x_hl", [2 * C, N], bf16, isOutput=False)
    xr_d = nc.declare_dram_parameter("x_res", [N, C], bf16, isOutput=False)
    # packed q|k weights, hi/lo stacked: [2C, 64] = [[wqT_hi wkT_hi]; [lo]]
    wqk_d = nc.declare_dram_parameter("wqkT_hl", [2 * C, 64], bf16, isOutput=False)
    wv_d = nc.declare_dram_parameter("wvT_h", [C, C], bf16, isOutput=False)
    bq_d = nc.declare_dram_parameter("bq", [C8, 1], f32, isOutput=False)
    gamma_d = nc.declare_dram_parameter("gamma", [1, 1], f32, isOutput=False)
    y_d = nc.declare_dram_parameter("y", [N, C], bf16, isOutput=True)

    with tile.TileContext(nc) as tc, ExitStack() as ctx:
        sing = ctx.enter_context(tc.tile_pool(name="sing", bufs=1))
        # e tiles live a full group (RND rounds) before AV consumes them
        epool = ctx.enter_context(tc.tile_pool(name="epool", bufs=RND + 2))
        ypool = ctx.enter_context(tc.tile_pool(name="ypool", bufs=2))
        scal = ctx.enter_context(tc.tile_pool(name="scal", bufs=4))

        lt_ps = ctx.enter_context(tc.tile_pool(name="lt_ps", bufs=2, space="PSUM"))
        u_ps = ctx.enter_context(tc.tile_pool(name="u_ps", bufs=1, space="PSUM"))

        for _rep in range(reps):
            # chunk 0 of x first (unblocks projections), then weights, then
            # the rest of x. slots (hl cc): 0=hi0, 1=hi1, 2=lo0, 3=lo1
            xhl_view = xhl_d[:].rearrange("(hl cc p) m -> p (hl cc) m", p=128, hl=2)
            # 4 chunks x 2 n-groups each, halves split across both HWDGE
            # rings (SP + ACT) - per-DMA fixed cost is ~0.6-2us serial per
            # ring, so fewer/bigger chunks win once projections overlap.
            CW = 2 * NG
            xhl_t = []
            for cidx in range(4):
                t = sing.tile([128, 4, CW], bf16)
                nc.sync.dma_start(
                    out=t[:, 0:2, :],
                    in_=xhl_view[:, 0:2, cidx * CW:(cidx + 1) * CW])
                nc.scalar.dma_start(
                    out=t[:, 2:4, :],
                    in_=xhl_view[:, 2:4, cidx * CW:(cidx + 1) * CW])
                xhl_t.append(t)
                if cidx == 0:
                    wqk_sb = sing.tile([128, 4, 64], bf16)
                    nc.sync.dma_start(
                        out=wqk_sb,
                        in_=wqk_d[:].rearrange("(hl cc p) o -> p (hl cc) o",
                                               p=128, hl=2))
                    wv_sb = sing.tile([128, 2, C], bf16)
                    nc.sync.dma_start(
                        out=wv_sb, in_=wv_d[:].rearrange("(cc p) c -> p cc c", p=128))
                    bq_sb = sing.tile([C8, 1], f32)
                    nc.sync.dma_start(out=bq_sb, in_=bq_d[:])
                    g128 = sing.tile([128, 1], f32)
                    nc.sync.dma_start(
                        out=g128,
                        in_=bass.AP(tensor=gamma_d, offset=0, ap=[[0, 128], [1, 1]]),
                    )

            # ---- q/k projections + vT, per chunk ----
            q_stack = sing.tile([3 * C8, N], bf16)
            k_stack = sing.tile([3 * C8, N], bf16)
            qlo_full = sing.tile([C8, N], bf16)
            klo_full = sing.tile([64, N], bf16)
            vt_sb = sing.tile([128, MC, CP], bf16)
            nc.vector.memset(vt_sb[:, :, C:CP], 1.0)   # Z ones-column
            term_slots = [0, 1, 2, 3, 0, 1]            # x slot per term
            w_slots = [0, 1, 0, 1, 2, 3]               # w slot per term
            for s in range(G):
                sl = slice(s * NG, (s + 1) * NG)
                csl = slice((s % 2) * NG, (s % 2 + 1) * NG)  # within-chunk cols
                xc = xhl_t[s // 2]
                pqk = u_ps.tile([64, NG], f32, tag=f"u{s % 2}", name="pqk")
                nterm = len(w_slots) if "lo" not in ablate else 2
                for i in range(nterm):
                    nc.tensor.matmul(pqk, wqk_sb[:, w_slots[i], :],
                                     xc[:, term_slots[i], csl],
                                     start=(i == 0), stop=(i == nterm - 1))
                # hi strips via ACT (bias fused for q); lo strips via one
                # fused DVE op each: lo = (pqk + b) - hi
                nc.scalar.activation(q_stack[0:C8, sl], pqk[0:C8, :],
                                     AF.Identity, bias=bq_sb)
                nc.scalar.activation(k_stack[C8:2 * C8, sl], pqk[C8:64, :], AF.Copy)
                nc.vector.scalar_tensor_tensor(
                    qlo_full[:, sl], pqk[0:C8, :], bq_sb, q_stack[0:C8, sl],
                    ALU.add, ALU.subtract)
                nc.vector.scalar_tensor_tensor(
                    klo_full[C8:64, sl], pqk[C8:64, :], 0.0,
                    k_stack[C8:2 * C8, sl], ALU.add, ALU.subtract)

                # vT for this chunk's 4 m-chunks (copies split DVE/ACT)
                for mc in range(4 * s, 4 * s + 4):
                    msl = slice((mc % 8) * 128, (mc % 8 + 1) * 128)
                    pv = u_ps.tile([128, C], f32, tag=f"u{2 + mc % 2}", name="pv")
                    for cc in range(2):
                        nc.tensor.matmul(pv, xc[:, cc, msl], wv_sb[:, cc, :],
                                         start=(cc == 0), stop=(cc == 1))
                    if mc % 2 == 0:
                        nc.vector.tensor_copy(vt_sb[:, mc, 0:C], pv)
                    else:
                        nc.scalar.activation(vt_sb[:, mc, 0:C], pv, AF.Copy)

            # strip duplication: 4 whole-row partition-shifting DMAs, split
            # across both rings (q on ACT, k on SP)
            nc.scalar.dma_start(out=q_stack[C8:2 * C8, :], in_=q_stack[0:C8, :])
            nc.scalar.dma_start(out=q_stack[2 * C8:3 * C8, :], in_=qlo_full)
            nc.sync.dma_start(out=k_stack[2 * C8:3 * C8, :],
                              in_=k_stack[C8:2 * C8, :])
            nc.sync.dma_start(out=k_stack[0:C8, :], in_=klo_full[C8:64, :])

            # residual chunks (first needed ~20us into attention)
            xr_t = []
            for h in range(2):
                tr = sing.tile([128, 16, C], bf16)
                nc.sync.dma_start(
                    out=tr,
                    in_=xr_d[:].rearrange("(gs p) c -> p gs c", p=128)[
                        :, h * 16:(h + 1) * 16, :])
                xr_t.append(tr)

            # ---- attention, software-pipelined one group deep ----
            e_tiles = {}
            u_tiles = {}

            def issue_lt_exp(g, j):
                sl = slice(g * NG, (g + 1) * NG)
                plt = lt_ps.tile([128, EW, NG], f32, tag="plt", name="plt")
                for rg in range(EW if "lt" not in ablate else 1):
                    mc = EW * j + rg
                    msl = slice(mc * 128, (mc + 1) * 128)
                    nc.tensor.matmul(plt[:, rg, :], k_stack[:, msl], q_stack[:, sl],
                                     start=True, stop=True)
                e_t = epool.tile([128, EW, NG], bf16, tag="e", name="e_t")
                fn = AF.Exp if "exp" not in ablate else AF.Copy
                nc.scalar.activation(e_t, plt, fn)
                e_tiles[(g, j)] = e_t

            def issue_av(g, j):
                uts = u_tiles[g]
                e_t = e_tiles.pop((g, j))
                if "av" in ablate:
                    if j == 0:
                        for sub in range(4):
                            nc.tensor.matmul(uts[sub],
                                             e_t[:, 0, sub * 128:(sub + 1) * 128],
                                             vt_sb[:, 0, :], start=True, stop=True)
                    return
                for rg in range(EW):
                    mc = EW * j + rg
                    first = (j == 0 and rg == 0)
                    last = (j == RND - 1 and rg == EW - 1)
                    for sub in range(4):
                        nc.tensor.matmul(uts[sub],
                                         e_t[:, rg, sub * 128:(sub + 1) * 128],
                                         vt_sb[:, mc, :],
                                         start=first, stop=last)

            def issue_epilogue(g):
                # DVE normalizes (gamma/Z); GPSIMD adds the bf16 residual
                uts = u_tiles.pop(g)
                y_g = ypool.tile([128, 4, C], bf16, tag="yg", name="y_g")
                for sub in range(4):
                    ut = uts[sub]
                    rinv = scal.tile([128, 1], f32, tag="rinv", name="rinv")
                    nc.vector.reciprocal(rinv, ut[:, C:CP])
                    gsc = scal.tile([128, 1], f32, tag="gsc", name="gsc")
                    nc.vector.tensor_scalar_mul(gsc, rinv, g128)
                    ysc = scal.tile([128, C], f32, tag="ysc", name="ysc")
                    nc.vector.tensor_scalar_mul(ysc, ut[:, 0:C], gsc)
                    nc.gpsimd.tensor_add(y_g[:, sub, :], ysc,
                                         xr_t[g // 4][:, (g % 4) * 4 + sub, :])
                nc.sync.dma_start(
                    out=y_d[:].rearrange("(gs p) c -> p gs c", p=128)[
                        :, g * 4:(g + 1) * 4, :],
                    in_=y_g,
                )

            for g in range(G + 1):
                if g < G:
                    u_tiles[g] = [u_ps.tile([128, CP], f32, tag=f"u{s}", name=f"u{s}")
                                  for s in range(4)]
                for j in range(RND):
                    if g < G:
                        issue_lt_exp(g, j)
                    if g >= 1:
                        issue_av(g - 1, j)
                if g >= 1:
                    issue_epilogue(g - 1)

    nc.compile()
    return nc


def prepare_in_maps(inputs):
    """Host-side prep: hi/lo splits, packed weights, residual fold."""
    import ml_dtypes
    bf = ml_dtypes.bfloat16
    x = np.asarray(inputs["x"], dtype=np.float32)
    wq = np.asarray(inputs["wq"], dtype=np.float32)
    bq = np.asarray(inputs["bq"], dtype=np.float32)
    wk = np.asarray(inputs["wk"], dtype=np.float32)
    wv = np.asarray(inputs["wv"], dtype=np.float32)
    bv = np.asarray(inputs["bv"], dtype=np.float32)
    gamma = np.asarray(inputs["gamma"], dtype=np.float32)

    xr = np.ascontiguousarray(x.reshape(B, C, N))
    x_hi = xr.astype(bf)
    x_lo = (xr - x_hi.astype(np.float32)).astype(bf)
    x_hl = np.concatenate([x_hi, x_lo], axis=1)  # [B, 2C, N]
    # residual in [N, C] layout with gamma*bv folded in (bf16: |err| ~0.4%)
    xres = np.ascontiguousarray(
        xr.transpose(0, 2, 1) + gamma[0] * bv[None, None, :]).astype(bf)

    wqkT = np.concatenate([wq.T, wk.T], axis=1)  # [C, 64]
    hi = wqkT.astype(bf)
    lo = (wqkT - hi.astype(np.float32)).astype(bf)
    wqkT_hl = np.ascontiguousarray(np.concatenate([hi, lo], axis=0))

    shared = {
        "wqkT_hl": wqkT_hl,
        "wvT_h": np.ascontiguousarray(wv.T.astype(bf)),
        "bq": np.ascontiguousarray(bq.reshape(C8, 1)),
        "gamma": np.ascontiguousarray(gamma.reshape(1, 1)),
    }
    return [dict(shared,
                 x_hl=np.ascontiguousarray(x_hl[i]),
                 x_res=xres[i]) for i in range(B)]


_nc_cache = None


def kernel(**inputs) -> np.ndarray:
    global _nc_cache
    if _nc_cache is None:
        _nc_cache = build_program()
    nc = _nc_cache

    in_maps = prepare_in_maps(inputs)
    res = run_bass_kernel_spmd(nc, in_maps, core_ids=list(range(B)))
    # y comes back [N, C] bf16 per core; transpose to [C, N] on host
    y = np.stack([res.results[i]["y"].astype(np.float32).T for i in range(B)],
                 axis=0)
    return np.ascontiguousarray(y.reshape(B, C, 64, 64))


if __name__ == "__main__":
    rng = np.random.default_rng(0)
    ins = {
        "x": rng.standard_normal((B, C, 64, 64), dtype=np.float32),
        "wq": rng.standard_normal((C8, C), dtype=np.float32) / 16,
        "bq": rng.standard_normal((C8,), dtype=np.float32) * 0.01,
        "wk": rng.standard_normal((C8, C), dtype=np.float32) / 16,
        "bk": rng.standard_normal((C8,), dtype=np.float32) * 0.01,
        "wv": rng.standard_normal((C, C), dtype=np.float32) / 16,
        "bv": rng.standard_normal((C,), dtype=np.float32) * 0.01,
        "gamma": rng.standard_normal((1,), dtype=np.float32) * 0.1,
    }
    out = kernel(**ins)
    print("kernel output", out.shape, out.dtype)


# revision 34
# speedup vs baseline: 1.0835x; 1.0094x over previous
"""Trainium2 Bass kernel for nn_Attention_7078106104284.

Self-attention block (SAGAN-style) over x[8, 256, 64, 64]:
  q = wq@x+bq [32,n], k = wk@x+bk [32,n], v = wv@x+bv [256,n], n = 4096
  attn = softmax(q^T k, axis=m);  y = x + gamma * (v @ attn^T)

Sharding: data-parallel over batch - one batch element per NeuronCore (8 cores).

Numerics: all matmuls bf16; q/k use a hi/lo split (a ~ a_hi + a_lo, both bf16)
so logits carry ~16 mantissa bits via K=96 stacked strips
  q_stack = [q_hi; q_hi; q_lo], k_stack = [k_lo; k_hi; k_hi].
Softmax max-subtraction skipped (|logit| < 50 << 88). Bias algebra:
  - bk drops entirely (q.bk is constant per softmax row -> cancels),
  - bq fuses into the q_hi evacuation on ACT,
  - bv folds into the residual (sum_m attn = 1), precomputed on host.

Dataflow per core (DMA count is minimized - each dma_start costs ~0.6us on
the issuing sequencer's HWDGE slot):
  - x arrives hi+lo packed, in 8 chunk DMAs; q/k projections start on chunk 0
    while later chunks stream (no startup DMA wall).
  - q+k share one stationary [128, 64]: each projection term is ONE matmul
    into pqk[64, 512] (q rows 0:32, k rows 32:64). Evac: ACT writes the hi
    strips (bq fused), DVE writes lo strips via fused (pqk+b)-hi.
  - vT[m, c'] via stationary x_hi chunks, interleaved with projections;
    Z ones-column via one strided memset (no K=1 matmuls).
  - strip duplication = 4 whole-row SBUF->SBUF DMAs after the loop.
  - attention: 16 rounds/group of (2 logit matmuls K=96 -> one fused exp on
    ACT over [128, 2*512] bf16 out -> 8 AV matmuls), plt double-buffered so
    logits(j+1) overlap exp(j); AV accumulates uT[n-sub, c'] in PSUM.
  - epilogue per sub: DVE normalize (gamma/Z), GPSIMD adds the residual
    (x^T + gamma*bv, bf16), one merged y DMA per group in [N, C] layout
    (host transposes back). No PE transposes anywhere.
"""

import sys

sys.path.insert(0, "/opt/trn_rl_repo")

import numpy as np
from contextlib import ExitStack

import concourse.bass as bass
import concourse.bacc as bacc
import concourse.tile as tile
import concourse.mybir as mybir
from concourse.bass_utils import run_bass_kernel_spmd

dt = mybir.dt
AF = mybir.ActivationFunctionType
ALU = mybir.AluOpType

B = 8
C = 256
C8 = 32
N = 4096          # h*w spatial positions
NG = 512          # n-group width (one PSUM bank of fp32)
G = N // NG       # 8 n-groups
MC = N // 128     # 32 m-chunks
EW = 2            # m-chunks per exp batch (PSUM banks per plt buffer)
RND = MC // EW    # 16 rounds per group
CP = C + 1        # AV output channels incl. the Z ones-column


def build_program(reps=1, ablate=(), dma_mode="split2"):
    nc = bacc.Bacc("TRN2", target_bir_lowering=False)
    f32 = dt.float32
    bf16 = dt.bfloat16
    # x_hl: [hl, C, N] = x_hi (hl=0) and x_lo (hl=1) packed in one tensor
    xhl_d = nc.declare_dram_parameter("x_hl", [2 * C, N], bf16, isOutput=False)
    xr_d = nc.declare_dram_parameter("x_res", [N, C], bf16, isOutput=False)
    # all bf16 weights in ONE tensor, partition-major:
    #   [:, 0:256] = wqkT_hl slots (hl cc, 64 each), [:, 256:768] = wvT (cc, 256)
    wpk_d = nc.declare_dram_parameter("wpack", [128, 768], bf16, isOutput=False)
    # col 0 = gamma replicated x128 by host, col 1 rows 0:32 = bq
    bqg_d = nc.declare_dram_parameter("bqg", [128, 2], f32, isOutput=False)
    y_d = nc.declare_dram_parameter("y", [N, C], bf16, isOutput=True)

    with tile.TileContext(nc) as tc, ExitStack() as ctx:
        sing = ctx.enter_context(tc.tile_pool(name="sing", bufs=1))
        xpool = ctx.enter_context(tc.tile_pool(name="xpool", bufs=1))
        # e tiles live a full group (RND rounds) before AV consumes them
        epool = ctx.enter_context(tc.tile_pool(name="epool", bufs=RND + 2))
        ypool = ctx.enter_context(tc.tile_pool(name="ypool", bufs=2))
        scal = ctx.enter_context(tc.tile_pool(name="scal", bufs=4))

        lt_ps = ctx.enter_context(tc.tile_pool(name="lt_ps", bufs=2, space="PSUM"))
        u_ps = ctx.enter_context(tc.tile_pool(name="u_ps", bufs=1, space="PSUM"))

        for _rep in range(reps):
            # chunk 0 of x first (unblocks projections), then weights, then
            # the rest of x. slots (hl cc): 0=hi0, 1=hi1, 2=lo0, 3=lo1
            xhl_view = xhl_d[:].rearrange("(hl cc p) m -> p (hl cc) m", p=128, hl=2)
            # x chunks (in n-groups): small first chunk so projections start
            # early; halves split across both HWDGE rings (SP + ACT).
            CHUNK_GROUPS = [[0], [1, 2], [3, 4], [5, 6, 7]]
            grp_chunk = {}
            grp_off = {}
            for ci, gs in enumerate(CHUNK_GROUPS):
                for oi, g_ in enumerate(gs):
                    grp_chunk[g_] = ci
                    grp_off[g_] = oi
            xhl_t = []
            for cidx, gs in enumerate(CHUNK_GROUPS):
                cw = len(gs) * NG
                c0 = gs[0] * NG
                t = xpool.tile([128, 4, cw], bf16, tag=f"x{cidx}", name=f"x{cidx}")
                nc.sync.dma_start(
                    out=t[:, 0:2, :], in_=xhl_view[:, 0:2, c0:c0 + cw])
                nc.scalar.dma_start(
                    out=t[:, 2:4, :], in_=xhl_view[:, 2:4, c0:c0 + cw])
                xhl_t.append(t)
                if cidx == 0:
                    wpk_sb = sing.tile([128, 768], bf16)
                    nc.sync.dma_start(out=wpk_sb, in_=wpk_d[:])
                    bqg_sb = sing.tile([128, 2], f32)
                    nc.sync.dma_start(out=bqg_sb, in_=bqg_d[:])
                    bq_sb = bqg_sb[0:C8, 1:2]
                    g128 = bqg_sb[:, 0:1]

            def wqk_slot(ws):
                return wpk_sb[:, ws * 64:(ws + 1) * 64]

            def wv_slot(cc):
                return wpk_sb[:, 256 + cc * C:256 + (cc + 1) * C]

            # ---- q/k projections + vT, per chunk ----
            q_stack = sing.tile([3 * C8, N], bf16)
            k_stack = sing.tile([3 * C8, N], bf16)
            qlo_full = sing.tile([C8, N], bf16)
            klo_full = sing.tile([64, N], bf16)
            vt_sb = sing.tile([128, MC, CP], bf16)
            nc.vector.memset(vt_sb[:, :, C:CP], 1.0)   # Z ones-column
            term_slots = [0, 1, 2, 3, 0, 1]            # x slot per term
            w_slots = [0, 1, 0, 1, 2, 3]               # w slot per term
            for s in range(G):
                sl = slice(s * NG, (s + 1) * NG)
                csl = slice(grp_off[s] * NG, (grp_off[s] + 1) * NG)
                xc = xhl_t[grp_chunk[s]]
                pqk = u_ps.tile([64, NG], f32, tag=f"u{s % 2}", name="pqk")
                nterm = len(w_slots) if "lo" not in ablate else 2
                for i in range(nterm):
                    nc.tensor.matmul(pqk, wqk_slot(w_slots[i]),
                                     xc[:, term_slots[i], csl],
                                     start=(i == 0), stop=(i == nterm - 1))
                # hi strips via ACT (bias fused for q); lo strips via one
                # fused DVE op each: lo = (pqk + b) - hi
                nc.scalar.activation(q_stack[0:C8, sl], pqk[0:C8, :],
                                     AF.Identity, bias=bq_sb)
                nc.scalar.activation(k_stack[C8:2 * C8, sl], pqk[C8:64, :], AF.Copy)
                nc.vector.scalar_tensor_tensor(
                    qlo_full[:, sl], pqk[0:C8, :], bq_sb, q_stack[0:C8, sl],
                    ALU.add, ALU.subtract)
                nc.vector.scalar_tensor_tensor(
                    klo_full[C8:64, sl], pqk[C8:64, :], 0.0,
                    k_stack[C8:2 * C8, sl], ALU.add, ALU.subtract)

                # vT for this group's 4 m-chunks (copies split DVE/ACT)
                for mc in range(4 * s, 4 * s + 4):
                    msl = slice((grp_off[s] * 4 + mc % 4) * 128,
                                (grp_off[s] * 4 + mc % 4 + 1) * 128)
                    pv = u_ps.tile([128, C], f32, tag=f"u{2 + mc % 2}", name="pv")
                    for cc in range(2):
                        nc.tensor.matmul(pv, xc[:, cc, msl], wv_slot(cc),
                                         start=(cc == 0), stop=(cc == 1))
                    if mc % 2 == 0:
                        nc.vector.tensor_copy(vt_sb[:, mc, 0:C], pv)
                    else:
                        nc.scalar.activation(vt_sb[:, mc, 0:C], pv, AF.Copy)

                # strip duplication for finished chunks, incrementally (q on
                # ACT ring, k on SP ring) so only the last chunk's dups sit
                # between projections and attention
                if s == CHUNK_GROUPS[grp_chunk[s]][-1]:
                    gs = CHUNK_GROUPS[grp_chunk[s]]
                    dsl = slice(gs[0] * NG, (gs[-1] + 1) * NG)
                    nc.scalar.dma_start(out=q_stack[C8:2 * C8, dsl],
                                        in_=q_stack[0:C8, dsl])
                    nc.scalar.dma_start(out=q_stack[2 * C8:3 * C8, dsl],
                                        in_=qlo_full[:, dsl])
                    nc.sync.dma_start(out=k_stack[2 * C8:3 * C8, dsl],
                                      in_=k_stack[C8:2 * C8, dsl])
                    nc.sync.dma_start(out=k_stack[0:C8, dsl],
                                      in_=klo_full[C8:64, dsl])

            # residual chunks (first needed ~20us into attention)
            xr_t = []
            for h in range(2):
                tr = sing.tile([128, 16, C], bf16, tag=f"xr{h}", name=f"xr{h}")
                nc.sync.dma_start(
                    out=tr,
                    in_=xr_d[:].rearrange("(gs p) c -> p gs c", p=128)[
                        :, h * 16:(h + 1) * 16, :])
                xr_t.append(tr)

            # ---- attention, software-pipelined one group deep ----
            e_tiles = {}
            u_tiles = {}

            def issue_lt_exp(g, j):
                sl = slice(g * NG, (g + 1) * NG)
                plt = lt_ps.tile([128, EW, NG], f32, tag="plt", name="plt")
                for rg in range(EW if "lt" not in ablate else 1):
                    mc = EW * j + rg
                    msl = slice(mc * 128, (mc + 1) * 128)
                    nc.tensor.matmul(plt[:, rg, :], k_stack[:, msl], q_stack[:, sl],
                                     start=True, stop=True)
                e_t = epool.tile([128, EW, NG], bf16, tag="e", name="e_t")
                fn = AF.Exp if "exp" not in ablate else AF.Copy
                nc.scalar.activation(e_t, plt, fn)
                e_tiles[(g, j)] = e_t

            def issue_av(g, j):
                uts = u_tiles[g]
                e_t = e_tiles.pop((g, j))
                if "av" in ablate:
                    if j == 0:
                        for sub in range(4):
                            nc.tensor.matmul(uts[sub],
                                             e_t[:, 0, sub * 128:(sub + 1) * 128],
                                             vt_sb[:, 0, :], start=True, stop=True)
                    return
                if j == RND - 1:
                    # last round sub-major: each sub's accumulation stops as
                    # early as possible so its epilogue overlaps remaining AV
                    for sub in range(4):
                        for rg in range(EW):
                            mc = EW * j + rg
                            nc.tensor.matmul(uts[sub],
                                             e_t[:, rg, sub * 128:(sub + 1) * 128],
                                             vt_sb[:, mc, :],
                                             start=False, stop=(rg == EW - 1))
                    return
                for rg in range(EW):
                    mc = EW * j + rg
                    first = (j == 0 and rg == 0)
                    for sub in range(4):
                        nc.tensor.matmul(uts[sub],
                                         e_t[:, rg, sub * 128:(sub + 1) * 128],
                                         vt_sb[:, mc, :],
                                         start=first, stop=False)

            def issue_epilogue(g):
                # DVE normalizes (gamma/Z); GPSIMD adds the bf16 residual
                uts = u_tiles.pop(g)
                y_g = ypool.tile([128, 4, C], bf16, tag="yg", name="y_g")
                for sub in range(4):
                    ut = uts[sub]
                    rinv = scal.tile([128, 1], f32, tag="rinv", name="rinv")
                    nc.vector.reciprocal(rinv, ut[:, C:CP])
                    gsc = scal.tile([128, 1], f32, tag="gsc", name="gsc")
                    nc.vector.tensor_scalar_mul(gsc, rinv, g128)
                    ysc = scal.tile([128, C], f32, tag="ysc", name="ysc")
                    nc.vector.tensor_scalar_mul(ysc, ut[:, 0:C], gsc)
                    nc.gpsimd.tensor_add(y_g[:, sub, :], ysc,
                                         xr_t[g // 4][:, (g % 4) * 4 + sub, :])
                nc.sync.dma_start(
                    out=y_d[:].rearrange("(gs p) c -> p gs c", p=128)[
                        :, g * 4:(g + 1) * 4, :],
                    in_=y_g,
                )

            for g in range(G + 1):
                if g < G:
                    u_tiles[g] = [u_ps.tile([128, CP], f32, tag=f"u{s}", name=f"u{s}")
                                  for s in range(4)]
                for j in range(RND):
                    if g < G:
                        issue_lt_exp(g, j)
                    if g >= 1:
                        issue_av(g - 1, j)
                if g >= 1:
                    issue_epilogue(g - 1)

    nc.compile()
    return nc


def prepare_in_maps(inputs):
    """Host-side prep: hi/lo splits, packed weights, residual fold."""
    import ml_dtypes
    bf = ml_dtypes.bfloat16
    x = np.asarray(inputs["x"], dtype=np.float32)
    wq = np.asarray(inputs["wq"], dtype=np.float32)
    bq = np.asarray(inputs["bq"], dtype=np.float32)
    wk = np.asarray(inputs["wk"], dtype=np.float32)
    wv = np.asarray(inputs["wv"], dtype=np.float32)
    bv = np.asarray(inputs["bv"], dtype=np.float32)
    gamma = np.asarray(inputs["gamma"], dtype=np.float32)

    xr = np.ascontiguousarray(x.reshape(B, C, N))
    x_hi = xr.astype(bf)
    x_lo = (xr - x_hi.astype(np.float32)).astype(bf)
    x_hl = np.concatenate([x_hi, x_lo], axis=1)  # [B, 2C, N]
    # residual in [N, C] layout with gamma*bv folded in (bf16: |err| ~0.4%)
    xres = np.ascontiguousarray(
        xr.transpose(0, 2, 1) + gamma[0] * bv[None, None, :]).astype(bf)

    wqkT = np.concatenate([wq.T, wk.T], axis=1)  # [C, 64]
    hi = wqkT.astype(bf)
    lo = (wqkT - hi.astype(np.float32)).astype(bf)
    wqkT_hl = np.concatenate([hi, lo], axis=0)   # [(hl cc p), 64]
    # pack all bf16 weights partition-major into [128, 768]:
    #   cols 0:256 = 4 slots of wqkT_hl, cols 256:768 = 2 cc-slices of wvT
    wqk_pm = wqkT_hl.reshape(4, 128, 64).transpose(1, 0, 2).reshape(128, 256)
    wvT = wv.T.astype(bf)                        # [(cc p), C]
    wv_pm = wvT.reshape(2, 128, C).transpose(1, 0, 2).reshape(128, 512)
    wpack = np.ascontiguousarray(np.concatenate([wqk_pm, wv_pm], axis=1))
    # bqg: col 0 = gamma replicated, col 1 rows 0:32 = bq
    bqg = np.zeros((128, 2), dtype=np.float32)
    bqg[:, 0] = gamma[0]
    bqg[0:C8, 1] = bq

    shared = {
        "wpack": wpack,
        "bqg": bqg,
    }
    return [dict(shared,
                 x_hl=np.ascontiguousarray(x_hl[i]),
                 x_res=xres[i]) for i in range(B)]


_nc_cache = None


def kernel(**inputs) -> np.ndarray:
    global _nc_cache
    if _nc_cache is None:
        _nc_cache = build_program()
    nc = _nc_cache

    in_maps = prepare_in_maps(inputs)
    res = run_bass_kernel_spmd(nc, in_maps, core_ids=list(range(B)))
    # y comes back [N, C] bf16 per core; transpose to [C, N] on host
    y = np.stack([res.results[i]["y"].astype(np.float32).T for i in range(B)],
                 axis=0)
    return np.ascontiguousarray(y.reshape(B, C, 64, 64))


if __name__ == "__main__":
    rng = np.random.default_rng(0)
    ins = {
        "x": rng.standard_normal((B, C, 64, 64), dtype=np.float32),
        "wq": rng.standard_normal((C8, C), dtype=np.float32) / 16,
        "bq": rng.standard_normal((C8,), dtype=np.float32) * 0.01,
        "wk": rng.standard_normal((C8, C), dtype=np.float32) / 16,
        "bk": rng.standard_normal((C8,), dtype=np.float32) * 0.01,
        "wv": rng.standard_normal((C, C), dtype=np.float32) / 16,
        "bv": rng.standard_normal((C,), dtype=np.float32) * 0.01,
        "gamma": rng.standard_normal((1,), dtype=np.float32) * 0.1,
    }
    out = kernel(**ins)
    print("kernel output", out.shape, out.dtype)


# revision 35
# speedup vs baseline: 1.2583x; 1.1614x over previous
"""Trainium2 Bass kernel for nn_Attention_7078106104284.

Self-attention block (SAGAN-style) over x[8, 256, 64, 64]:
  q = wq@x+bq [32,n], k = wk@x+bk [32,n], v = wv@x+bv [256,n], n = 4096
  attn = softmax(q^T k, axis=m);  y = x + gamma * (v @ attn^T)

Sharding: data-parallel over batch - one batch element per NeuronCore (8 cores).

Numerics: plain bf16 matmuls throughout (fp32 PSUM accumulation). Measured on
the actual task data, logit-path hi/lo splits change the final error not at
all - the bf16 output/residual quantization (~0.6% of out-scale, vs the 2%
gate) dominates. Bias algebra:
  - bk drops entirely (q.bk is constant per softmax row -> cancels),
  - bq fuses into the q evacuation on ACT,
  - bv folds into the residual (sum_m attn = 1), precomputed on host.
Softmax max-subtraction skipped (|logit| < 50 << 88; exp and Z ride in
f32/bf16 range).

Dataflow per core (DMA count kept low - each dma_start costs ~0.6-2us of
serial ring time; loop-allocated tiles get distinct pool tags so their DMAs
are not serialized behind the previous tile's consumers):
  - x (bf16) arrives in 4 chunks (1+2+2+3 n-groups), each split across both
    HWDGE rings (SP + ACT); projections start on chunk 0 immediately.
  - q+k share one stationary [128, 64]: each chunk-group needs just TWO
    accumulating matmuls into pqk[64, 512] (q rows 0:32, k rows 32:64).
    ACT evacuates q (bias fused) into q_rep[0:32] and k into k_rep[32:64];
    one partition-shift DMA per chunk completes each replica pair.
  - vT[m, c'] via stationary x chunks, interleaved with projections; the
    Z ones-column is one strided memset; copies on DVE.
  - attention, one group deep in software pipeline: per round TWO logit
    matmuls (K=32) issued at row_grp 0 and 32 via base-partition-derived
    tile_position - the PE array runs them CONCURRENTLY in different 32-row
    strips -> one fused exp on ACT over [128, 2*512] bf16 -> 8 AV matmuls
    accumulating uT[n-sub, c'] (c'=256 ones column carries Z).
  - epilogue per sub: DVE normalize (gamma/Z), GPSIMD adds the residual
    (x^T + gamma*bv, bf16), one merged y DMA per group in [N, C] layout
    (host transposes back). No PE transposes anywhere.
"""

import sys

sys.path.insert(0, "/opt/trn_rl_repo")

import numpy as np
from contextlib import ExitStack

import concourse.bass as bass
import concourse.bacc as bacc
import concourse.tile as tile
import concourse.mybir as mybir
from concourse.bass_utils import run_bass_kernel_spmd

dt = mybir.dt
AF = mybir.ActivationFunctionType

B = 8
C = 256
C8 = 32
N = 4096          # h*w spatial positions
NG = 512          # n-group width (one PSUM bank of fp32)
G = N // NG       # 8 n-groups
MC = N // 128     # 32 m-chunks
EW = 2            # m-chunks per exp batch (PSUM banks per plt buffer)
RND = MC // EW    # 16 rounds per group
CP = C + 1        # AV output channels incl. the Z ones-column


def build_program(reps=1, ablate=()):
    nc = bacc.Bacc("TRN2", target_bir_lowering=False)
    f32 = dt.float32
    bf16 = dt.bfloat16
    xh_d = nc.declare_dram_parameter("x_h", [C, N], bf16, isOutput=False)
    xr_d = nc.declare_dram_parameter("x_res", [N, C], bf16, isOutput=False)
    # all bf16 weights in ONE tensor, partition-major:
    #   [:, 0:128] = wqkT cc-slots (64 each), [:, 128:640] = wvT (cc, 256)
    wpk_d = nc.declare_dram_parameter("wpack", [128, 640], bf16, isOutput=False)
    # col 0 = gamma replicated x128 by host, col 1 rows 0:32 = bq
    bqg_d = nc.declare_dram_parameter("bqg", [128, 2], f32, isOutput=False)
    y_d = nc.declare_dram_parameter("y", [N, C], bf16, isOutput=True)

    with tile.TileContext(nc) as tc, ExitStack() as ctx:
        sing = ctx.enter_context(tc.tile_pool(name="sing", bufs=1))
        xpool = ctx.enter_context(tc.tile_pool(name="xpool", bufs=1))
        # e tiles live a full group (RND rounds) before AV consumes them
        epool = ctx.enter_context(tc.tile_pool(name="epool", bufs=RND + 2))
        ypool = ctx.enter_context(tc.tile_pool(name="ypool", bufs=2))
        scal = ctx.enter_context(tc.tile_pool(name="scal", bufs=4))

        lt_ps = ctx.enter_context(tc.tile_pool(name="lt_ps", bufs=2, space="PSUM"))
        u_ps = ctx.enter_context(tc.tile_pool(name="u_ps", bufs=1, space="PSUM"))

        for _rep in range(reps):
            xh_view = xh_d[:].rearrange("(cc p) m -> p cc m", p=128)
            # x chunks (in n-groups): small first chunk so projections start
            # early; halves split across both HWDGE rings (SP + ACT).
            CHUNK_GROUPS = [[0], [1, 2], [3, 4], [5, 6, 7]]
            grp_chunk = {}
            grp_off = {}
            for ci, gs in enumerate(CHUNK_GROUPS):
                for oi, g_ in enumerate(gs):
                    grp_chunk[g_] = ci
                    grp_off[g_] = oi
            xh_t = []
            for cidx, gs in enumerate(CHUNK_GROUPS):
                cw = len(gs) * NG
                c0 = gs[0] * NG
                t = xpool.tile([128, 2, cw], bf16, tag=f"x{cidx}", name=f"x{cidx}")
                nc.sync.dma_start(out=t[:, 0:1, :], in_=xh_view[:, 0:1, c0:c0 + cw])
                nc.scalar.dma_start(out=t[:, 1:2, :], in_=xh_view[:, 1:2, c0:c0 + cw])
                xh_t.append(t)
                if cidx == 0:
                    wpk_sb = sing.tile([128, 640], bf16)
                    nc.sync.dma_start(out=wpk_sb, in_=wpk_d[:])
                    bqg_sb = sing.tile([128, 2], f32)
                    nc.sync.dma_start(out=bqg_sb, in_=bqg_d[:])
                    bq_sb = bqg_sb[0:C8, 1:2]
                    g128 = bqg_sb[:, 0:1]

            def wqk_slot(cc):
                return wpk_sb[:, cc * 64:(cc + 1) * 64]

            def wv_slot(cc):
                return wpk_sb[:, 128 + cc * C:128 + (cc + 1) * C]

            # ---- q/k projections + vT, per group ----
            # q_rep/k_rep hold q (k) on BOTH partition strips 0:32 and 32:64
            # so logit matmuls can be issued row-tiled at row_grp 0 and 32.
            q_rep = sing.tile([64, N], bf16)
            k_rep = sing.tile([64, N], bf16)
            vt_sb = sing.tile([128, MC, CP], bf16)
            nc.vector.memset(vt_sb[:, :, C:CP], 1.0)   # Z ones-column
            for s in range(G):
                sl = slice(s * NG, (s + 1) * NG)
                csl = slice(grp_off[s] * NG, (grp_off[s] + 1) * NG)
                xc = xh_t[grp_chunk[s]]
                pqk = u_ps.tile([64, NG], f32, tag=f"u{s % 2}", name="pqk")
                for cc in range(2):
                    nc.tensor.matmul(pqk, wqk_slot(cc), xc[:, cc, csl],
                                     start=(cc == 0), stop=(cc == 1))
                # ACT evacuation: q with fused bias, k plain (lane-aligned)
                nc.scalar.activation(q_rep[0:C8, sl], pqk[0:C8, :],
                                     AF.Identity, bias=bq_sb)
                nc.scalar.activation(k_rep[C8:64, sl], pqk[C8:64, :], AF.Copy)

                # vT for this group's 4 m-chunks (copies on DVE)
                for mc in range(4 * s, 4 * s + 4):
                    msl = slice((grp_off[s] * 4 + mc % 4) * 128,
                                (grp_off[s] * 4 + mc % 4 + 1) * 128)
                    pv = u_ps.tile([128, C], f32, tag=f"u{2 + mc % 2}", name="pv")
                    for cc in range(2):
                        nc.tensor.matmul(pv, xc[:, cc, msl], wv_slot(cc),
                                         start=(cc == 0), stop=(cc == 1))
                    nc.vector.tensor_copy(vt_sb[:, mc, 0:C], pv)

                # replica completion per finished chunk (q on ACT ring, k on
                # SP ring): one partition-shift DMA each
                if s == CHUNK_GROUPS[grp_chunk[s]][-1]:
                    gs = CHUNK_GROUPS[grp_chunk[s]]
                    dsl = slice(gs[0] * NG, (gs[-1] + 1) * NG)
                    nc.scalar.dma_start(out=q_rep[C8:64, dsl],
                                        in_=q_rep[0:C8, dsl])
                    nc.sync.dma_start(out=k_rep[0:C8, dsl],
                                      in_=k_rep[C8:64, dsl])

            # residual chunks (first needed ~20us into attention)
            xr_t = []
            for h in range(2):
                tr = sing.tile([128, 16, C], bf16, tag=f"xr{h}", name=f"xr{h}")
                nc.sync.dma_start(
                    out=tr,
                    in_=xr_d[:].rearrange("(gs p) c -> p gs c", p=128)[
                        :, h * 16:(h + 1) * 16, :])
                xr_t.append(tr)

            # ---- attention, software-pipelined one group deep ----
            e_tiles = {}
            u_tiles = {}

            def issue_lt_exp(g, j):
                sl = slice(g * NG, (g + 1) * NG)
                plt = lt_ps.tile([128, EW, NG], f32, tag="plt", name="plt")
                for rg in range(EW if "lt" not in ablate else 1):
                    mc = EW * j + rg
                    msl = slice(mc * 128, (mc + 1) * 128)
                    r0, r1 = rg * C8, (rg + 1) * C8
                    # row_grp = 32*rg (auto-derived from base partition):
                    # the two K=32 matmuls run concurrently in the PE array
                    nc.tensor.matmul(plt[:, rg, :], k_rep[r0:r1, msl],
                                     q_rep[r0:r1, sl], start=True, stop=True)
                e_t = epool.tile([128, EW, NG], bf16, tag="e", name="e_t")
                fn = AF.Exp if "exp" not in ablate else AF.Copy
                nc.scalar.activation(e_t, plt, fn)
                e_tiles[(g, j)] = e_t

            def issue_av(g, j):
                uts = u_tiles[g]
                e_t = e_tiles.pop((g, j))
                if "av" in ablate:
                    if j == 0:
                        for sub in range(4):
                            nc.tensor.matmul(uts[sub],
                                             e_t[:, 0, sub * 128:(sub + 1) * 128],
                                             vt_sb[:, 0, :], start=True, stop=True)
                    return
                if j == RND - 1:
                    # last round sub-major: each sub's accumulation stops as
                    # early as possible so its epilogue overlaps remaining AV
                    for sub in range(4):
                        for rg in range(EW):
                            mc = EW * j + rg
                            nc.tensor.matmul(uts[sub],
                                             e_t[:, rg, sub * 128:(sub + 1) * 128],
                                             vt_sb[:, mc, :],
                                             start=False, stop=(rg == EW - 1))
                    return
                for rg in range(EW):
                    mc = EW * j + rg
                    first = (j == 0 and rg == 0)
                    for sub in range(4):
                        nc.tensor.matmul(uts[sub],
                                         e_t[:, rg, sub * 128:(sub + 1) * 128],
                                         vt_sb[:, mc, :],
                                         start=first, stop=False)

            def issue_epilogue(g):
                # DVE normalizes (gamma/Z); GPSIMD adds the bf16 residual
                uts = u_tiles.pop(g)
                y_g = ypool.tile([128, 4, C], bf16, tag="yg", name="y_g")
                for sub in range(4):
                    ut = uts[sub]
                    rinv = scal.tile([128, 1], f32, tag="rinv", name="rinv")
                    nc.vector.reciprocal(rinv, ut[:, C:CP])
                    gsc = scal.tile([128, 1], f32, tag="gsc", name="gsc")
                    nc.vector.tensor_scalar_mul(gsc, rinv, g128)
                    ysc = scal.tile([128, C], f32, tag="ysc", name="ysc")
                    nc.vector.tensor_scalar_mul(ysc, ut[:, 0:C], gsc)
                    nc.gpsimd.tensor_add(y_g[:, sub, :], ysc,
                                         xr_t[g // 4][:, (g % 4) * 4 + sub, :])
                nc.sync.dma_start(
                    out=y_d[:].rearrange("(gs p) c -> p gs c", p=128)[
                        :, g * 4:(g + 1) * 4, :],
                    in_=y_g,
                )

            for g in range(G + 1):
                if g < G:
                    u_tiles[g] = [u_ps.tile([128, CP], f32, tag=f"u{s}", name=f"u{s}")
                                  for s in range(4)]
                for j in range(RND):
                    if g < G:
                        issue_lt_exp(g, j)
                    if g >= 1:
                        issue_av(g - 1, j)
                if g >= 1:
                    issue_epilogue(g - 1)

    nc.compile()
    return nc


def prepare_in_maps(inputs):
    """Host-side prep: bf16 casts, packed weights, residual fold."""
    import ml_dtypes
    bf = ml_dtypes.bfloat16
    x = np.asarray(inputs["x"], dtype=np.float32)
    wq = np.asarray(inputs["wq"], dtype=np.float32)
    bq = np.asarray(inputs["bq"], dtype=np.float32)
    wk = np.asarray(inputs["wk"], dtype=np.float32)
    wv = np.asarray(inputs["wv"], dtype=np.float32)
    bv = np.asarray(inputs["bv"], dtype=np.float32)
    gamma = np.asarray(inputs["gamma"], dtype=np.float32)

    xr = np.ascontiguousarray(x.reshape(B, C, N))
    x_h = xr.astype(bf)
    # residual in [N, C] layout with gamma*bv folded in (bf16: |err| ~0.4%)
    xres = np.ascontiguousarray(
        xr.transpose(0, 2, 1) + gamma[0] * bv[None, None, :]).astype(bf)

    # pack all bf16 weights partition-major into [128, 640]:
    #   cols 0:128 = 2 cc-slots of wqkT ([wq.T wk.T]), cols 128:640 = wvT
    wqkT = np.concatenate([wq.T, wk.T], axis=1).astype(bf)   # [(cc p), 64]
    wqk_pm = wqkT.reshape(2, 128, 64).transpose(1, 0, 2).reshape(128, 128)
    wvT = wv.T.astype(bf)                                    # [(cc p), C]
    wv_pm = wvT.reshape(2, 128, C).transpose(1, 0, 2).reshape(128, 512)
    wpack = np.ascontiguousarray(np.concatenate([wqk_pm, wv_pm], axis=1))
    # bqg: col 0 = gamma replicated, col 1 rows 0:32 = bq
    bqg = np.zeros((128, 2), dtype=np.float32)
    bqg[:, 0] = gamma[0]
    bqg[0:C8, 1] = bq

    shared = {"wpack": wpack, "bqg": bqg}
    return [dict(shared,
                 x_h=np.ascontiguousarray(x_h[i]),
                 x_res=xres[i]) for i in range(B)]


_nc_cache = None


def kernel(**inputs) -> np.ndarray:
    global _nc_cache
    if _nc_cache is None:
        _nc_cache = build_program()
    nc = _nc_cache

    in_maps = prepare_in_maps(inputs)
    res = run_bass_kernel_spmd(nc, in_maps, core_ids=list(range(B)))
    # y comes back [N, C] bf16 per core; transpose to [C, N] on host
    y = np.stack([res.results[i]["y"].astype(np.float32).T for i in range(B)],
                 axis=0)
    return np.ascontiguousarray(y.reshape(B, C, 64, 64))


if __name__ == "__main__":
    rng = np.random.default_rng(0)
    ins = {
        "x": rng.standard_normal((B, C, 64, 64), dtype=np.float32),
        "wq": rng.standard_normal((C8, C), dtype=np.float32) / 16,
        "bq": rng.standard_normal((C8,), dtype=np.float32) * 0.01,
        "wk": rng.standard_normal((C8, C), dtype=np.float32) / 16,
        "bk": rng.standard_normal((C8,), dtype=np.float32) * 0.01,
        "wv": rng.standard_normal((C, C), dtype=np.float32) / 16,
        "bv": rng.standard_normal((C,), dtype=np.float32) * 0.01,
        "gamma": rng.standard_normal((1,), dtype=np.float32) * 0.1,
    }
    out = kernel(**ins)
    print("kernel output", out.shape, out.dtype)


# revision 38
# speedup vs baseline: 1.3098x; 1.0409x over previous
"""Trainium2 Bass kernel for nn_Attention_7078106104284.

Self-attention block (SAGAN-style) over x[8, 256, 64, 64]:
  q = wq@x+bq [32,n], k = wk@x+bk [32,n], v = wv@x+bv [256,n], n = 4096
  attn = softmax(q^T k, axis=m);  y = x + gamma * (v @ attn^T)

Sharding: data-parallel over batch - one batch element per NeuronCore (8 cores).

Numerics: plain bf16 matmuls throughout (fp32 PSUM accumulation). Measured on
the actual task data, logit-path hi/lo splits change the final error not at
all - the bf16 output/residual quantization (~0.6% of out-scale, vs the 2%
gate) dominates. Bias algebra:
  - bk drops entirely (q.bk is constant per softmax row -> cancels),
  - bq fuses into the q evacuation on ACT,
  - bv folds into the residual (sum_m attn = 1), precomputed on host.
Softmax max-subtraction skipped (|logit| < 50 << 88; exp and Z ride in
f32/bf16 range).

Dataflow per core (DMA count kept low - each dma_start costs ~0.6-2us of
serial ring time; loop-allocated tiles get distinct pool tags so their DMAs
are not serialized behind the previous tile's consumers):
  - x (bf16) arrives in 4 chunks (1+2+2+3 n-groups), each split across both
    HWDGE rings (SP + ACT); projections start on chunk 0 immediately.
  - q+k share one stationary [128, 64]: each chunk-group needs just TWO
    accumulating matmuls into pqk[64, 512] (q rows 0:32, k rows 32:64).
    ACT evacuates q (bias fused) into q_rep[0:32] and k into k_rep[32:64];
    one partition-shift DMA per chunk completes each replica pair.
  - vT[m, c'] via stationary x chunks, interleaved with projections; the
    Z ones-column is one strided memset; copies on DVE.
  - attention, one group deep in software pipeline: per round TWO logit
    matmuls (K=32) issued at row_grp 0 and 32 via base-partition-derived
    tile_position - the PE array runs them CONCURRENTLY in different 32-row
    strips -> one fused exp on ACT over [128, 2*512] bf16 -> 8 AV matmuls
    accumulating uT[n-sub, c'] (c'=256 ones column carries Z).
  - epilogue per sub: DVE normalize (gamma/Z), GPSIMD adds the residual
    (x^T + gamma*bv, bf16), one merged y DMA per group in [N, C] layout
    (host transposes back). No PE transposes anywhere.
"""

import sys

sys.path.insert(0, "/opt/trn_rl_repo")

import numpy as np
from contextlib import ExitStack

import concourse.bass as bass
import concourse.bacc as bacc
import concourse.tile as tile
import concourse.mybir as mybir
from concourse.bass_utils import run_bass_kernel_spmd

dt = mybir.dt
AF = mybir.ActivationFunctionType

B = 8
C = 256
C8 = 32
N = 4096          # h*w spatial positions
NG = 512          # n-group width (one PSUM bank of fp32)
G = N // NG       # 8 n-groups
MC = N // 128     # 32 m-chunks
EW = 2            # m-chunks per exp batch (PSUM banks per plt buffer)
RND = MC // EW    # 16 rounds per group
CP = C + 1        # AV output channels incl. the Z ones-column


def build_program(reps=1, ablate=()):
    nc = bacc.Bacc("TRN2", target_bir_lowering=False)
    f32 = dt.float32
    bf16 = dt.bfloat16
    xh_d = nc.declare_dram_parameter("x_h", [C, N], bf16, isOutput=False)
    xr_d = nc.declare_dram_parameter("x_res", [N, C], bf16, isOutput=False)
    # all bf16 weights in ONE tensor, partition-major:
    #   [:, 0:128] = wqkT cc-slots (64 each), [:, 128:640] = wvT (cc, 256)
    wpk_d = nc.declare_dram_parameter("wpack", [128, 640], bf16, isOutput=False)
    # col 0 = gamma replicated x128 by host, col 1 rows 0:32 = bq
    bqg_d = nc.declare_dram_parameter("bqg", [128, 2], f32, isOutput=False)
    y_d = nc.declare_dram_parameter("y", [N, C], bf16, isOutput=True)

    with tile.TileContext(nc) as tc, ExitStack() as ctx:
        sing = ctx.enter_context(tc.tile_pool(name="sing", bufs=1))
        xpool = ctx.enter_context(tc.tile_pool(name="xpool", bufs=1))
        # e tiles live a full group (RND rounds) before AV consumes them
        epool = ctx.enter_context(tc.tile_pool(name="epool", bufs=RND + 2))
        ypool = ctx.enter_context(tc.tile_pool(name="ypool", bufs=2))
        scal = ctx.enter_context(tc.tile_pool(name="scal", bufs=4))

        lt_ps = ctx.enter_context(tc.tile_pool(name="lt_ps", bufs=2, space="PSUM"))
        u_ps = ctx.enter_context(tc.tile_pool(name="u_ps", bufs=1, space="PSUM"))

        for _rep in range(reps):
            xh_view = xh_d[:].rearrange("(cc p) m -> p cc m", p=128)
            # x chunks (in n-groups): small first chunk so projections start
            # early; halves split across both HWDGE rings (SP + ACT).
            CHUNK_GROUPS = [[0], [1, 2], [3, 4], [5, 6, 7]]
            grp_chunk = {}
            grp_off = {}
            for ci, gs in enumerate(CHUNK_GROUPS):
                for oi, g_ in enumerate(gs):
                    grp_chunk[g_] = ci
                    grp_off[g_] = oi
            xh_t = []
            for cidx, gs in enumerate(CHUNK_GROUPS):
                cw = len(gs) * NG
                c0 = gs[0] * NG
                t = xpool.tile([128, 2, cw], bf16, tag=f"x{cidx}", name=f"x{cidx}")
                nc.sync.dma_start(out=t[:, 0:1, :], in_=xh_view[:, 0:1, c0:c0 + cw])
                nc.scalar.dma_start(out=t[:, 1:2, :], in_=xh_view[:, 1:2, c0:c0 + cw])
                xh_t.append(t)
                if cidx == 0:
                    wpk_sb = sing.tile([128, 640], bf16)
                    nc.sync.dma_start(out=wpk_sb, in_=wpk_d[:])
                    bqg_sb = sing.tile([128, 2], f32)
                    nc.sync.dma_start(out=bqg_sb, in_=bqg_d[:])
                    bq_sb = bqg_sb[0:C8, 1:2]
                    g128 = bqg_sb[:, 0:1]

            def wqk_slot(cc):
                return wpk_sb[:, cc * 64:(cc + 1) * 64]

            def wv_slot(cc):
                return wpk_sb[:, 128 + cc * C:128 + (cc + 1) * C]

            # ---- q/k projections + vT, per group ----
            # q_rep/k_rep hold q (k) on BOTH partition strips 0:32 and 32:64
            # so logit matmuls can be issued row-tiled at row_grp 0 and 32.
            q_rep = sing.tile([64, N], bf16)
            k_rep = sing.tile([64, N], bf16)
            vt_sb = sing.tile([128, MC, CP], bf16)
            nc.vector.memset(vt_sb[:, :, C:CP], 1.0)   # Z ones-column
            for s in range(G):
                sl = slice(s * NG, (s + 1) * NG)
                csl = slice(grp_off[s] * NG, (grp_off[s] + 1) * NG)
                xc = xh_t[grp_chunk[s]]
                pqk = u_ps.tile([64, NG], f32, tag=f"u{s % 2}", name="pqk")
                for cc in range(2):
                    nc.tensor.matmul(pqk, wqk_slot(cc), xc[:, cc, csl],
                                     start=(cc == 0), stop=(cc == 1))
                # ACT evacuation: q with fused bias, k plain (lane-aligned)
                nc.scalar.activation(q_rep[0:C8, sl], pqk[0:C8, :],
                                     AF.Identity, bias=bq_sb)
                nc.scalar.activation(k_rep[C8:64, sl], pqk[C8:64, :], AF.Copy)

                # vT for this group's 4 m-chunks (copies on DVE)
                for mc in range(4 * s, 4 * s + 4):
                    msl = slice((grp_off[s] * 4 + mc % 4) * 128,
                                (grp_off[s] * 4 + mc % 4 + 1) * 128)
                    pv = u_ps.tile([128, C], f32, tag=f"u{2 + mc % 2}", name="pv")
                    for cc in range(2):
                        nc.tensor.matmul(pv, xc[:, cc, msl], wv_slot(cc),
                                         start=(cc == 0), stop=(cc == 1))
                    nc.vector.tensor_copy(vt_sb[:, mc, 0:C], pv)

                # replica completion per finished chunk (q on ACT ring, k on
                # SP ring): one partition-shift DMA each
                if s == CHUNK_GROUPS[grp_chunk[s]][-1]:
                    gs = CHUNK_GROUPS[grp_chunk[s]]
                    dsl = slice(gs[0] * NG, (gs[-1] + 1) * NG)
                    nc.scalar.dma_start(out=q_rep[C8:64, dsl],
                                        in_=q_rep[0:C8, dsl])
                    nc.sync.dma_start(out=k_rep[0:C8, dsl],
                                      in_=k_rep[C8:64, dsl])

            # residual chunks (first needed ~20us into attention)
            xr_t = []
            for h in range(2):
                tr = sing.tile([128, 16, C], bf16, tag=f"xr{h}", name=f"xr{h}")
                nc.sync.dma_start(
                    out=tr,
                    in_=xr_d[:].rearrange("(gs p) c -> p gs c", p=128)[
                        :, h * 16:(h + 1) * 16, :])
                xr_t.append(tr)

            # ---- attention, software-pipelined one group deep ----
            e_tiles = {}
            u_tiles = {}

            def issue_lt_exp(g, j):
                sl = slice(g * NG, (g + 1) * NG)
                plt = lt_ps.tile([128, EW, NG], f32, tag="plt", name="plt")
                for rg in range(EW if "lt" not in ablate else 1):
                    mc = EW * j + rg
                    msl = slice(mc * 128, (mc + 1) * 128)
                    r0, r1 = rg * C8, (rg + 1) * C8
                    # row_grp = 32*rg (auto-derived from base partition):
                    # the two K=32 matmuls run concurrently in the PE array
                    nc.tensor.matmul(plt[:, rg, :], k_rep[r0:r1, msl],
                                     q_rep[r0:r1, sl], start=True, stop=True)
                e_t = epool.tile([128, EW, NG], bf16, tag="e", name="e_t")
                fn = AF.Exp if "exp" not in ablate else AF.Copy
                nc.scalar.activation(e_t, plt, fn)
                e_tiles[(g, j)] = e_t

            def issue_av(g, j):
                uts = u_tiles[g]
                e_t = e_tiles.pop((g, j))
                if "av" in ablate:
                    if j == 0:
                        for sub in range(4):
                            nc.tensor.matmul(uts[sub],
                                             e_t[:, 0, sub * 128:(sub + 1) * 128],
                                             vt_sb[:, 0, :], start=True, stop=True)
                    return
                if j == RND - 1:
                    # last round sub-major: each sub's accumulation stops as
                    # early as possible so its epilogue overlaps remaining AV
                    for sub in range(4):
                        for rg in range(EW):
                            mc = EW * j + rg
                            nc.tensor.matmul(uts[sub],
                                             e_t[:, rg, sub * 128:(sub + 1) * 128],
                                             vt_sb[:, mc, :],
                                             start=False, stop=(rg == EW - 1))
                    return
                for rg in range(EW):
                    mc = EW * j + rg
                    first = (j == 0 and rg == 0)
                    for sub in range(4):
                        nc.tensor.matmul(uts[sub],
                                         e_t[:, rg, sub * 128:(sub + 1) * 128],
                                         vt_sb[:, mc, :],
                                         start=first, stop=False)

            def issue_epilogue(g):
                # DVE normalizes (gamma/Z); GPSIMD adds the bf16 residual.
                # Last group streams each sub's y out separately (short tail).
                uts = u_tiles.pop(g)
                y_view = y_d[:].rearrange("(gs p) c -> p gs c", p=128)
                y_g = ypool.tile([128, 4, C], bf16, tag="yg", name="y_g")
                for sub in range(4):
                    ut = uts[sub]
                    rinv = scal.tile([128, 1], f32, tag="rinv", name="rinv")
                    nc.vector.reciprocal(rinv, ut[:, C:CP])
                    gsc = scal.tile([128, 1], f32, tag="gsc", name="gsc")
                    nc.vector.tensor_scalar_mul(gsc, rinv, g128)
                    ysc = scal.tile([128, C], f32, tag="ysc", name="ysc")
                    nc.vector.tensor_scalar_mul(ysc, ut[:, 0:C], gsc)
                    nc.gpsimd.tensor_add(y_g[:, sub, :], ysc,
                                         xr_t[g // 4][:, (g % 4) * 4 + sub, :])
                    if g == G - 1:
                        nc.sync.dma_start(out=y_view[:, g * 4 + sub, :],
                                          in_=y_g[:, sub, :])
                if g < G - 1:
                    nc.sync.dma_start(out=y_view[:, g * 4:(g + 1) * 4, :],
                                      in_=y_g)

            for g in range(G + 1):
                if g < G:
                    u_tiles[g] = [u_ps.tile([128, CP], f32, tag=f"u{s}", name=f"u{s}")
                                  for s in range(4)]
                for j in range(RND):
                    if g < G:
                        issue_lt_exp(g, j)
                    if g >= 1:
                        issue_av(g - 1, j)
                if g >= 1:
                    issue_epilogue(g - 1)

    nc.compile()
    return nc


def prepare_in_maps(inputs):
    """Host-side prep: bf16 casts, packed weights, residual fold."""
    import ml_dtypes
    bf = ml_dtypes.bfloat16
    x = np.asarray(inputs["x"], dtype=np.float32)
    wq = np.asarray(inputs["wq"], dtype=np.float32)
    bq = np.asarray(inputs["bq"], dtype=np.float32)
    wk = np.asarray(inputs["wk"], dtype=np.float32)
    wv = np.asarray(inputs["wv"], dtype=np.float32)
    bv = np.asarray(inputs["bv"], dtype=np.float32)
    gamma = np.asarray(inputs["gamma"], dtype=np.float32)

    xr = np.ascontiguousarray(x.reshape(B, C, N))
    x_h = xr.astype(bf)
    # residual in [N, C] layout with gamma*bv folded in (bf16: |err| ~0.4%)
    xres = np.ascontiguousarray(
        xr.transpose(0, 2, 1) + gamma[0] * bv[None, None, :]).astype(bf)

    # pack all bf16 weights partition-major into [128, 640]:
    #   cols 0:128 = 2 cc-slots of wqkT ([wq.T wk.T]), cols 128:640 = wvT
    wqkT = np.concatenate([wq.T, wk.T], axis=1).astype(bf)   # [(cc p), 64]
    wqk_pm = wqkT.reshape(2, 128, 64).transpose(1, 0, 2).reshape(128, 128)
    wvT = wv.T.astype(bf)                                    # [(cc p), C]
    wv_pm = wvT.reshape(2, 128, C).transpose(1, 0, 2).reshape(128, 512)
    wpack = np.ascontiguousarray(np.concatenate([wqk_pm, wv_pm], axis=1))
    # bqg: col 0 = gamma replicated, col 1 rows 0:32 = bq
    bqg = np.zeros((128, 2), dtype=np.float32)
    bqg[:, 0] = gamma[0]
    bqg[0:C8, 1] = bq

    shared = {"wpack": wpack, "bqg": bqg}
    return [dict(shared,
                 x_h=np.ascontiguousarray(x_h[i]),
                 x_res=xres[i]) for i in range(B)]


_nc_cache = None


def kernel(**inputs) -> np.ndarray:
    global _nc_cache
    if _nc_cache is None:
        _nc_cache = build_program()
    nc = _nc_cache

    in_maps = prepare_in_maps(inputs)
    res = run_bass_kernel_spmd(nc, in_maps, core_ids=list(range(B)))
    # y comes back [N, C] bf16 per core; transpose to [C, N] on host
    y = np.stack([res.results[i]["y"].astype(np.float32).T for i in range(B)],
                 axis=0)
    return np.ascontiguousarray(y.reshape(B, C, 64, 64))


if __name__ == "__main__":
    rng = np.random.default_rng(0)
    ins = {
        "x": rng.standard_normal((B, C, 64, 64), dtype=np.float32),
        "wq": rng.standard_normal((C8, C), dtype=np.float32) / 16,
        "bq": rng.standard_normal((C8,), dtype=np.float32) * 0.01,
        "wk": rng.standard_normal((C8, C), dtype=np.float32) / 16,
        "bk": rng.standard_normal((C8,), dtype=np.float32) * 0.01,
        "wv": rng.standard_normal((C, C), dtype=np.float32) / 16,
        "bv": rng.standard_normal((C,), dtype=np.float32) * 0.01,
        "gamma": rng.standard_normal((1,), dtype=np.float32) * 0.1,
    }
    out = kernel(**ins)
    print("kernel output", out.shape, out.dtype)


# revision 43
# speedup vs baseline: 1.3799x; 1.0535x over previous
"""Trainium2 Bass kernel for nn_Attention_7078106104284.

Self-attention block (SAGAN-style) over x[8, 256, 64, 64]:
  q = wq@x+bq [32,n], k = wk@x+bk [32,n], v = wv@x+bv [256,n], n = 4096
  attn = softmax(q^T k, axis=m);  y = x + gamma * (v @ attn^T)

Sharding: data-parallel over batch - one batch element per NeuronCore (8 cores).

Numerics: plain bf16 matmuls throughout (fp32 PSUM accumulation). Measured on
the actual task data, logit-path hi/lo splits change the final error not at
all - the bf16 output/residual quantization (~0.6% of out-scale, vs the 2%
gate) dominates. Bias algebra:
  - bk drops entirely (q.bk is constant per softmax row -> cancels),
  - bq fuses into the q evacuation on ACT,
  - bv folds into the residual (sum_m attn = 1), precomputed on host.
Softmax max-subtraction skipped (|logit| < 50 << 88; exp and Z ride in
f32/bf16 range).

Dataflow per core (DMA count kept low - each dma_start costs ~0.6-2us of
serial ring time; loop-allocated tiles get distinct pool tags so their DMAs
are not serialized behind the previous tile's consumers):
  - x (bf16) arrives in 4 chunks (1+2+2+3 n-groups), each split across both
    HWDGE rings (SP + ACT); projections start on chunk 0 immediately.
  - q+k share one stationary [128, 64]: each chunk-group needs just TWO
    accumulating matmuls into pqk[64, 512] (q rows 0:32, k rows 32:64).
    ACT evacuates q (bias fused) into q_rep[0:32] and k into k_rep[32:64];
    one partition-shift DMA per chunk completes each replica pair.
  - vT[m, c'] via stationary x chunks, interleaved with projections; the
    Z ones-column is one strided memset; copies on DVE.
  - attention, one group deep in software pipeline: per round TWO logit
    matmuls (K=32) issued at row_grp 0 and 32 via base-partition-derived
    tile_position - the PE array runs them CONCURRENTLY in different 32-row
    strips -> one fused exp on ACT over [128, 2*512] bf16 -> 8 AV matmuls
    accumulating uT[n-sub, c'] (c'=256 ones column carries Z).
  - epilogue per sub: DVE normalize (gamma/Z), GPSIMD adds the residual
    (x^T + gamma*bv, bf16), one merged y DMA per group in [N, C] layout
    (host transposes back). No PE transposes anywhere.
"""

import sys

sys.path.insert(0, "/opt/trn_rl_repo")

import numpy as np
from contextlib import ExitStack

import concourse.bass as bass
import concourse.bacc as bacc
import concourse.tile as tile
import concourse.mybir as mybir
from concourse.bass_utils import run_bass_kernel_spmd

dt = mybir.dt
AF = mybir.ActivationFunctionType

B = 8
C = 256
C8 = 32
N = 4096          # h*w spatial positions
NG = 512          # n-group width (one PSUM bank of fp32)
G = N // NG       # 8 n-groups
MC = N // 128     # 32 m-chunks
EW = 2            # m-chunks per exp batch (PSUM banks per plt buffer)
RND = MC // EW    # 16 rounds per group
CP = C + 1        # AV output channels incl. the Z ones-column


def build_program(reps=1, ablate=()):
    nc = bacc.Bacc("TRN2", target_bir_lowering=False)
    f32 = dt.float32
    bf16 = dt.bfloat16
    xh_d = nc.declare_dram_parameter("x_h", [C, N], bf16, isOutput=False)
    xr_d = nc.declare_dram_parameter("x_res", [N, C], bf16, isOutput=False)
    # all bf16 weights in ONE tensor, partition-major:
    #   [:, 0:128] = wqkT cc-slots (64 each), [:, 128:640] = wvT (cc, 256)
    wpk_d = nc.declare_dram_parameter("wpack", [128, 640], bf16, isOutput=False)
    # col 0 = gamma replicated x128 by host, col 1 rows 0:32 = bq
    bqg_d = nc.declare_dram_parameter("bqg", [128, 2], f32, isOutput=False)
    y_d = nc.declare_dram_parameter("y", [N, C], bf16, isOutput=True)

    with tile.TileContext(nc) as tc, ExitStack() as ctx:
        sing = ctx.enter_context(tc.tile_pool(name="sing", bufs=1))
        xpool = ctx.enter_context(tc.tile_pool(name="xpool", bufs=1))
        # e tiles live a full group (RND rounds) before AV consumes them
        epool = ctx.enter_context(tc.tile_pool(name="epool", bufs=RND + 2))
        ypool = ctx.enter_context(tc.tile_pool(name="ypool", bufs=2))
        scal = ctx.enter_context(tc.tile_pool(name="scal", bufs=4))

        lt_ps = ctx.enter_context(tc.tile_pool(name="lt_ps", bufs=2, space="PSUM"))
        u_ps = ctx.enter_context(tc.tile_pool(name="u_ps", bufs=1, space="PSUM"))

        for _rep in range(reps):
            xh_view = xh_d[:].rearrange("(cc p) m -> p cc m", p=128)
            # x chunks (in n-groups): small first chunk so projections start
            # early; halves split across both HWDGE rings (SP + ACT).
            CHUNK_GROUPS = [[0], [1, 2], [3, 4], [5, 6, 7]]
            grp_chunk = {}
            grp_off = {}
            for ci, gs in enumerate(CHUNK_GROUPS):
                for oi, g_ in enumerate(gs):
                    grp_chunk[g_] = ci
                    grp_off[g_] = oi
            xh_t = []
            for cidx, gs in enumerate(CHUNK_GROUPS):
                cw = len(gs) * NG
                c0 = gs[0] * NG
                t = xpool.tile([128, 2, cw], bf16, tag=f"x{cidx}", name=f"x{cidx}")
                nc.sync.dma_start(out=t[:, 0:1, :], in_=xh_view[:, 0:1, c0:c0 + cw])
                nc.scalar.dma_start(out=t[:, 1:2, :], in_=xh_view[:, 1:2, c0:c0 + cw])
                xh_t.append(t)
                if cidx == 0:
                    wpk_sb = sing.tile([128, 640], bf16)
                    nc.sync.dma_start(out=wpk_sb, in_=wpk_d[:])
                    bqg_sb = sing.tile([128, 2], f32)
                    nc.sync.dma_start(out=bqg_sb, in_=bqg_d[:])
                    bq_sb = bqg_sb[0:C8, 1:2]
                    g128 = bqg_sb[:, 0:1]

            def wqk_slot(cc):
                return wpk_sb[:, cc * 64:(cc + 1) * 64]

            def wv_slot(cc):
                return wpk_sb[:, 128 + cc * C:128 + (cc + 1) * C]

            # ---- q/k projections + vT, per group ----
            # q_rep/k_rep hold q (k) on ALL FOUR partition strips so logit
            # matmuls can be issued row-tiled at row_grp 0/32/64/96 - the PE
            # array runs four K=32 matmuls concurrently.
            q_rep = sing.tile([128, N], bf16)
            k_rep = sing.tile([128, N], bf16)
            vt_sb = sing.tile([128, MC, CP], bf16)
            nc.vector.memset(vt_sb[:, :, C:CP], 1.0)   # Z ones-column
            for s in range(G):
                sl = slice(s * NG, (s + 1) * NG)
                csl = slice(grp_off[s] * NG, (grp_off[s] + 1) * NG)
                xc = xh_t[grp_chunk[s]]
                pqk = u_ps.tile([64, NG], f32, tag=f"u{s % 2}", name="pqk")
                for cc in range(2):
                    nc.tensor.matmul(pqk, wqk_slot(cc), xc[:, cc, csl],
                                     start=(cc == 0), stop=(cc == 1))
                # ACT evacuation: q with fused bias, k plain (lane-aligned)
                nc.scalar.activation(q_rep[0:C8, sl], pqk[0:C8, :],
                                     AF.Identity, bias=bq_sb)
                nc.scalar.activation(k_rep[C8:64, sl], pqk[C8:64, :], AF.Copy)

                # vT for this group's 4 m-chunks (copies on DVE)
                for mc in range(4 * s, 4 * s + 4):
                    msl = slice((grp_off[s] * 4 + mc % 4) * 128,
                                (grp_off[s] * 4 + mc % 4 + 1) * 128)
                    pv = u_ps.tile([128, C], f32, tag=f"u{2 + mc % 2}", name="pv")
                    for cc in range(2):
                        nc.tensor.matmul(pv, xc[:, cc, msl], wv_slot(cc),
                                         start=(cc == 0), stop=(cc == 1))
                    nc.vector.tensor_copy(vt_sb[:, mc, 0:C], pv)

                # replica completion per finished chunk (q on ACT ring, k on
                # SP ring): one replicating partition-shift DMA each fans the
                # natural strip out to the other three
                if s == CHUNK_GROUPS[grp_chunk[s]][-1]:
                    gs = CHUNK_GROUPS[grp_chunk[s]]
                    dsl = slice(gs[0] * NG, (gs[-1] + 1) * NG)
                    nc.scalar.dma_start(out=q_rep[C8:64, dsl],
                                        in_=q_rep[0:C8, dsl])
                    nc.scalar.dma_start(out=q_rep[64:128, dsl],
                                        in_=q_rep[0:64, dsl])
                    nc.sync.dma_start(out=k_rep[0:C8, dsl],
                                      in_=k_rep[C8:64, dsl])
                    nc.sync.dma_start(out=k_rep[64:128, dsl],
                                      in_=k_rep[0:64, dsl])

            # residual chunks (first needed ~20us into attention)
            xr_t = []
            for h in range(2):
                tr = sing.tile([128, 16, C], bf16, tag=f"xr{h}", name=f"xr{h}")
                nc.sync.dma_start(
                    out=tr,
                    in_=xr_d[:].rearrange("(gs p) c -> p gs c", p=128)[
                        :, h * 16:(h + 1) * 16, :])
                xr_t.append(tr)

            # ---- attention, software-pipelined one group deep ----
            e_tiles = {}
            u_tiles = {}

            def issue_lt_exp(g, j):
                # rounds are emitted in pairs: even j uses row_grps 0/32,
                # odd j 64/96, so four K=32 logit matmuls sit back-to-back in
                # the PE queue and run concurrently in the four 32-row strips
                sl = slice(g * NG, (g + 1) * NG)
                plt = lt_ps.tile([128, EW, NG], f32, tag="plt", name="plt")
                for rg in range(EW if "lt" not in ablate else 1):
                    mc = EW * j + rg
                    msl = slice(mc * 128, (mc + 1) * 128)
                    row = ((j % 2) * 2 + rg) * C8
                    tp = (96, 0) if row == 96 else None
                    nc.tensor.matmul(plt[:, rg, :], k_rep[row:row + C8, msl],
                                     q_rep[row:row + C8, sl],
                                     start=True, stop=True, tile_position=tp)
                e_t = epool.tile([128, EW, NG], bf16, tag="e", name="e_t")
                fn = AF.Exp if "exp" not in ablate else AF.Copy
                nc.scalar.activation(e_t, plt, fn)
                e_tiles[(g, j)] = e_t

            def issue_av(g, j):
                uts = u_tiles[g]
                e_t = e_tiles.pop((g, j))
                if "av" in ablate:
                    if j == 0:
                        for sub in range(4):
                            nc.tensor.matmul(uts[sub],
                                             e_t[:, 0, sub * 128:(sub + 1) * 128],
                                             vt_sb[:, 0, :], start=True, stop=True)
                    return
                if j == RND - 1:
                    # last round sub-major: each sub's accumulation stops as
                    # early as possible so its epilogue overlaps remaining AV
                    for sub in range(4):
                        for rg in range(EW):
                            mc = EW * j + rg
                            nc.tensor.matmul(uts[sub],
                                             e_t[:, rg, sub * 128:(sub + 1) * 128],
                                             vt_sb[:, mc, :],
                                             start=False, stop=(rg == EW - 1))
                    return
                for rg in range(EW):
                    mc = EW * j + rg
                    first = (j == 0 and rg == 0)
                    for sub in range(4):
                        nc.tensor.matmul(uts[sub],
                                         e_t[:, rg, sub * 128:(sub + 1) * 128],
                                         vt_sb[:, mc, :],
                                         start=first, stop=False)

            def issue_epilogue(g):
                # DVE normalizes (gamma/Z); GPSIMD adds the bf16 residual.
                # Last group streams each sub's y out separately (short tail).
                uts = u_tiles.pop(g)
                y_view = y_d[:].rearrange("(gs p) c -> p gs c", p=128)
                y_g = ypool.tile([128, 4, C], bf16, tag="yg", name="y_g")
                for sub in range(4):
                    ut = uts[sub]
                    rinv = scal.tile([128, 1], f32, tag="rinv", name="rinv")
                    nc.vector.reciprocal(rinv, ut[:, C:CP])
                    gsc = scal.tile([128, 1], f32, tag="gsc", name="gsc")
                    nc.vector.tensor_scalar_mul(gsc, rinv, g128)
                    ysc = scal.tile([128, C], f32, tag="ysc", name="ysc")
                    nc.vector.tensor_scalar_mul(ysc, ut[:, 0:C], gsc)
                    nc.gpsimd.tensor_add(y_g[:, sub, :], ysc,
                                         xr_t[g // 4][:, (g % 4) * 4 + sub, :])
                    if g == G - 1:
                        nc.sync.dma_start(out=y_view[:, g * 4 + sub, :],
                                          in_=y_g[:, sub, :])
                if g < G - 1:
                    nc.sync.dma_start(out=y_view[:, g * 4:(g + 1) * 4, :],
                                      in_=y_g)

            for g in range(G + 1):
                if g < G:
                    u_tiles[g] = [u_ps.tile([128, CP], f32, tag=f"u{s}", name=f"u{s}")
                                  for s in range(4)]
                for jp in range(RND // 2):
                    if g < G:
                        issue_lt_exp(g, 2 * jp)
                        issue_lt_exp(g, 2 * jp + 1)
                    if g >= 1:
                        issue_av(g - 1, 2 * jp)
                        issue_av(g - 1, 2 * jp + 1)
                if g >= 1:
                    issue_epilogue(g - 1)

    nc.compile()
    return nc


def prepare_in_maps(inputs):
    """Host-side prep: bf16 casts, packed weights, residual fold."""
    import ml_dtypes
    bf = ml_dtypes.bfloat16
    x = np.asarray(inputs["x"], dtype=np.float32)
    wq = np.asarray(inputs["wq"], dtype=np.float32)
    bq = np.asarray(inputs["bq"], dtype=np.float32)
    wk = np.asarray(inputs["wk"], dtype=np.float32)
    wv = np.asarray(inputs["wv"], dtype=np.float32)
    bv = np.asarray(inputs["bv"], dtype=np.float32)
    gamma = np.asarray(inputs["gamma"], dtype=np.float32)

    xr = np.ascontiguousarray(x.reshape(B, C, N))
    x_h = xr.astype(bf)
    # residual in [N, C] layout with gamma*bv folded in (bf16: |err| ~0.4%)
    xres = np.ascontiguousarray(
        xr.transpose(0, 2, 1) + gamma[0] * bv[None, None, :]).astype(bf)

    # pack all bf16 weights partition-major into [128, 640]:
    #   cols 0:128 = 2 cc-slots of wqkT ([wq.T wk.T]), cols 128:640 = wvT
    wqkT = np.concatenate([wq.T, wk.T], axis=1).astype(bf)   # [(cc p), 64]
    wqk_pm = wqkT.reshape(2, 128, 64).transpose(1, 0, 2).reshape(128, 128)
    wvT = wv.T.astype(bf)                                    # [(cc p), C]
    wv_pm = wvT.reshape(2, 128, C).transpose(1, 0, 2).reshape(128, 512)
    wpack = np.ascontiguousarray(np.concatenate([wqk_pm, wv_pm], axis=1))
    # bqg: col 0 = gamma replicated, col 1 rows 0:32 = bq
    bqg = np.zeros((128, 2), dtype=np.float32)
    bqg[:, 0] = gamma[0]
    bqg[0:C8, 1] = bq

    shared = {"wpack": wpack, "bqg": bqg}
    return [dict(shared,
                 x_h=np.ascontiguousarray(x_h[i]),
                 x_res=xres[i]) for i in range(B)]


_nc_cache = None


def kernel(**inputs) -> np.ndarray:
    global _nc_cache
    if _nc_cache is None:
        _nc_cache = build_program()
    nc = _nc_cache

    in_maps = prepare_in_maps(inputs)
    res = run_bass_kernel_spmd(nc, in_maps, core_ids=list(range(B)))
    # y comes back [N, C] bf16 per core; transpose to [C, N] on host
    y = np.stack([res.results[i]["y"].astype(np.float32).T for i in range(B)],
                 axis=0)
    return np.ascontiguousarray(y.reshape(B, C, 64, 64))


if __name__ == "__main__":
    rng = np.random.default_rng(0)
    ins = {
        "x": rng.standard_normal((B, C, 64, 64), dtype=np.float32),
        "wq": rng.standard_normal((C8, C), dtype=np.float32) / 16,
        "bq": rng.standard_normal((C8,), dtype=np.float32) * 0.01,
        "wk": rng.standard_normal((C8, C), dtype=np.float32) / 16,
        "bk": rng.standard_normal((C8,), dtype=np.float32) * 0.01,
        "wv": rng.standard_normal((C, C), dtype=np.float32) / 16,
        "bv": rng.standard_normal((C,), dtype=np.float32) * 0.01,
        "gamma": rng.standard_normal((1,), dtype=np.float32) * 0.1,
    }
    out = kernel(**ins)
    print("kernel output", out.shape, out.dtype)
